# revision 6
# baseline (speedup 1.0000x reference)
"""Trainium2 Bass kernel for nn_KinematicOperation (kinematic tree forward).

Structure of the (deterministic) problem instance:
  - N = 1 + 2048*768 + 2048*256 atoms.
  - gen0: 2048 chains of 768 atoms rooted at the virtual root (identity HT);
    chain atoms are contiguous: chain c = atoms [1+c*768, 1+(c+1)*768).
  - gen1: 2048 branches of 256 atoms rooted mid-chain (gen0 chain c position
    384); branch atoms contiguous starting at boff = 1 + 2048*768.
  - Local HTs: BOND everywhere except a JUMP at each chain start; root = I.
  - Output: coords[id_idx[a-1]] = prefix_HT(a)[:3, 3] for atoms a = 1..N-1.

Sharding: core k owns gen0 chains [256k, 256(k+1)) and gen1 branches of the
same index range, so the branch-root HT handoff between generations stays
on-core and no collectives are needed.  Host pre-slices bond dof columns
(0..3 of 9) and gathers jump rows, shrinking input DMA.

Device algorithm per generation (fp32; rotations stored as 3x3 row-major,
translations separately):
  - ACT computes sin/cos (one DVE range-wrap per angle; cos as
    sin(pi/2 - |w|)); DVE assembles the local 3x3 rotations into SBUF.
  - 3-level blocked prefix scan along each chain:
      level1: rotation-only scan propagating ROWS 0,1 (6 elems) in place;
      translations via the NeRF identity local_t = d * col0(localR):
      t_glob(p) = sum_{q<=p} d_q * col0(R_glob_q), so in-block translations
      are prefix SUMS of w = d * col0(R_inblock) (col0 z-comp from a cross
      product), then level2/3 compose full 3x4 block HTs (tiny), and the
      final transform applies block-exclusive R,t to the in-block cumsums.
  - Output xyz written scatter-ready; host applies the id_idx permutation.
"""

import os
import sys

import numpy as np

for _p in ("/opt/trn_rl_repo", "/root/.axon_site/_ro/trn_rl_repo"):
    if os.path.isdir(_p) and _p not in sys.path:
        sys.path.insert(0, _p)

# ---------------------------------------------------------------- constants
C0, L0 = 2048, 768
C1, L1 = 2048, 256
N = 1 + C0 * L0 + C1 * L1
BOFF = 1 + C0 * L0
NCORES = 8
P = 128
CHI = 2                      # chains per partition (256 chains per core)
CH0 = C0 // NCORES
CH1 = C1 // NCORES
A0 = CH0 * L0                # 196608 gen0 atoms per core
A1 = CH1 * L1                # 65536 gen1 atoms per core

# block geometry: L = T*J,  J = S*U supers x blocks
T0, J0, S0, U0 = 12, 64, 8, 8
F0 = CHI * J0                # 128 block-lanes per partition
T1, J1, S1, U1 = 8, 32, 4, 8
F1 = CHI * J1                # 64

PI = float(np.pi)

_CACHE = {}


# ------------------------------------------------------------- device build
def _build_program(repeat=1):
    from concourse import bacc, mybir, tile
    from concourse.bass import AP

    f32 = mybir.dt.float32
    MUL = mybir.AluOpType.mult
    SUB = mybir.AluOpType.subtract
    SIN = mybir.ActivationFunctionType.Sin
    ABS = mybir.ActivationFunctionType.Abs

    nc = bacc.Bacc("TRN2", target_bir_lowering=False, debug=False)

    b0_d = nc.dram_tensor("b0", [A0, 4], f32, kind="ExternalInput")
    b1_d = nc.dram_tensor("b1", [A1, 4], f32, kind="ExternalInput")
    jd_d = nc.dram_tensor("jd", [P, CHI * 9], f32, kind="ExternalInput")
    kin0_d = nc.dram_tensor("kin0", [P, F0 * T0 * 3], f32, kind="ExternalOutput")
    kin1_d = nc.dram_tensor("kin1", [P, F1 * T1 * 3], f32, kind="ExternalOutput")

    def apx(tl, off, *dims):
        """AP over tile-AP `tl` at free-elem offset `off` with free dims
        [(step, count), ...] (full 128 partitions)."""
        t = tl[:] if not isinstance(tl, AP) else tl
        return AP(t.tensor, t.offset + off, [[t.ap[0][0], P]] + [list(d) for d in dims])

    def compose_1d(vec, lanes, a_off, a_step, b_off, b_step, o_off, o_step,
                   tA, tB, a_tile, b_tile, o_tile):
        """C = A @ B (3x4 HT compose, 12-elem row-major layout) over lanes."""
        for k, dst in ((0, tA), (1, tB)):
            vec.tensor_mul(
                out=apx(dst, 0, (12, lanes), (4, 3), (1, 4)),
                in0=apx(a_tile, a_off + k, (a_step, lanes), (4, 3), (0, 4)),
                in1=apx(b_tile, b_off + 4 * k, (b_step, lanes), (0, 3), (1, 4)),
            )
        vec.tensor_add(
            out=apx(tA, 0, (12, lanes), (1, 12)),
            in0=apx(tA, 0, (12, lanes), (1, 12)),
            in1=apx(tB, 0, (12, lanes), (1, 12)))
        vec.tensor_mul(
            out=apx(tB, 0, (12, lanes), (4, 3), (1, 4)),
            in0=apx(a_tile, a_off + 2, (a_step, lanes), (4, 3), (0, 4)),
            in1=apx(b_tile, b_off + 8, (b_step, lanes), (0, 3), (1, 4)),
        )
        vec.tensor_add(
            out=apx(o_tile, o_off, (o_step, lanes), (1, 12)),
            in0=apx(tA, 0, (12, lanes), (1, 12)),
            in1=apx(tB, 0, (12, lanes), (1, 12)),
        )
        vec.tensor_add(
            out=apx(o_tile, o_off + 3, (o_step, lanes), (4, 3)),
            in0=apx(o_tile, o_off + 3, (o_step, lanes), (4, 3)),
            in1=apx(a_tile, a_off + 3, (a_step, lanes), (4, 3)),
        )

    def excl_blocks(vec, CS, U, LPS, spx, lp2, rx, tA, tB):
        """rx[cs, u] = spx[cs] @ lp2[cs, u]  (exclusive block prefixes)."""
        for i in range(3):
            for k, dst in ((0, tA), (1, tB)):
                vec.tensor_mul(
                    out=apx(dst, 4 * i, (96, CS), (12, U), (1, 4)),
                    in0=apx(spx, 4 * i + k, (12, CS), (0, U), (0, 4)),
                    in1=apx(lp2, 4 * k, (LPS, CS), (12, U), (1, 4)))
            vec.tensor_add(
                out=apx(tA, 4 * i, (96, CS), (12, U), (1, 4)),
                in0=apx(tA, 4 * i, (96, CS), (12, U), (1, 4)),
                in1=apx(tB, 4 * i, (96, CS), (12, U), (1, 4)))
            vec.tensor_mul(
                out=apx(tB, 4 * i, (96, CS), (12, U), (1, 4)),
                in0=apx(spx, 4 * i + 2, (12, CS), (0, U), (0, 4)),
                in1=apx(lp2, 8, (LPS, CS), (12, U), (1, 4)))
            vec.tensor_add(
                out=apx(rx, 4 * i, (96, CS), (12, U), (1, 4)),
                in0=apx(tA, 4 * i, (96, CS), (12, U), (1, 4)),
                in1=apx(tB, 4 * i, (96, CS), (12, U), (1, 4)))
        vec.tensor_add(
            out=apx(rx, 3, (96, CS), (12, U), (4, 3)),
            in0=apx(rx, 3, (96, CS), (12, U), (4, 3)),
            in1=apx(spx, 3, (12, CS), (0, U), (4, 3)))

    # ---- generation emitters (engine-parameterized, f-lane-ranged) ----
    # Lane f = chi*J + j indexes a block; chain position = j*T + t, and the
    # per-atom source index chi*L + j*T + t == f*T + t, so every per-atom
    # phase uses 2-D (f, t) access patterns and splits at any f boundary.

    def emit_trig(V, S, dof, trig, aw, L, halfpi):
        """Wrap + sin/cos planes for angle cols 0,1,3 of the 4-wide dofs.
        One DVE range-wrap per angle (into the cos plane as scratch), sin on
        ACT, cos = sin(pi/2 - |w|) on ACT."""
        for col, cosn, sinn in ((0, "cp", "sp"), (1, "ct", "st"),
                                (3, "cc", "sc")):
            src = apx(dof, col, (L * 4, CHI), (4, L))
            V.add_range_wrap(out=trig[cosn][:], in_=src, shift=0.0,
                             bound=PI, period=2 * PI)
            S.activation(out=trig[sinn][:], in_=trig[cosn][:], func=SIN)
            S.activation(out=aw[:], in_=trig[cosn][:], func=ABS)
            S.activation(out=trig[cosn][:], in_=aw[:], func=SIN,
                         scale=-1.0, bias=halfpi[:])

    def emit_bond(V, stt, trig, X, T, F, f0, nf):
        """Local 3x3 bond rotations into X slabs for lanes [f0, f0+nf)."""
        def ti(nm):
            return apx(trig[nm], f0 * T, (T, nf), (1, T))

        def xo(e):
            return apx(X, f0 * 9 + e, (9, nf), (F * 9, T))

        tm1, tm2, tu, tv = (trig["tm1"], trig["tm2"], trig["tu"], trig["tv"])

        def tt(tl):
            return apx(tl, f0 * T, (T, nf), (1, T))

        V.tensor_scalar_mul(out=xo(0), in0=ti("ct"), scalar1=-1.0)       # -ct
        stt(out=xo(1), in0=ti("st"), scalar=-1.0, in1=ti("cc"),
            op0=MUL, op1=MUL)                                            # -st*cc
        V.tensor_mul(out=xo(2), in0=ti("st"), in1=ti("sc"))              # st*sc
        V.tensor_mul(out=xo(3), in0=ti("cp"), in1=ti("st"))              # cp*st
        V.tensor_mul(out=tt(tm1), in0=ti("cp"), in1=ti("ct"))            # u
        V.tensor_mul(out=tt(tm2), in0=ti("sp"), in1=ti("ct"))            # v
        V.tensor_mul(out=tt(tu), in0=tt(tm1), in1=ti("cc"))
        V.tensor_mul(out=tt(tv), in0=ti("sp"), in1=ti("sc"))
        stt(out=xo(4), in0=tt(tu), scalar=-1.0, in1=tt(tv),
            op0=MUL, op1=SUB)                                            # -u*cc-sp*sc
        V.tensor_mul(out=tt(tu), in0=tt(tm1), in1=ti("sc"))
        V.tensor_mul(out=tt(tv), in0=ti("sp"), in1=ti("cc"))
        V.tensor_sub(out=xo(5), in0=tt(tu), in1=tt(tv))                  # u*sc-sp*cc
        V.tensor_mul(out=xo(6), in0=ti("sp"), in1=ti("st"))              # sp*st
        V.tensor_mul(out=tt(tu), in0=tt(tm2), in1=ti("cc"))
        V.tensor_mul(out=tt(tv), in0=ti("cp"), in1=ti("sc"))
        V.tensor_sub(out=xo(7), in0=tt(tv), in1=tt(tu))                  # cp*sc-v*cc
        V.tensor_mul(out=tt(tu), in0=tt(tm2), in1=ti("sc"))
        V.tensor_mul(out=tt(tv), in0=ti("cp"), in1=ti("cc"))
        V.tensor_add(out=xo(8), in0=tt(tu), in1=tt(tv))                  # v*sc+cp*cc

    def emit_scan(V, X, tA, tB, T, F, f0, nf):
        """In-place in-block scan of rotation rows 0,1 for lanes [f0,f0+nf)
        (state in X slab t, elems 0..5; local row2 in elems 6..8 stays)."""
        for t in range(1, T):
            pb = (t - 1) * F * 9 + f0 * 9
            cb = t * F * 9 + f0 * 9
            V.tensor_mul(out=apx(tA, 0, (6, nf), (3, 2), (1, 3)),
                         in0=apx(X, pb + 0, (9, nf), (3, 2), (0, 3)),
                         in1=apx(X, cb + 0, (9, nf), (0, 2), (1, 3)))
            V.tensor_mul(out=apx(tB, 0, (6, nf), (3, 2), (1, 3)),
                         in0=apx(X, pb + 1, (9, nf), (3, 2), (0, 3)),
                         in1=apx(X, cb + 3, (9, nf), (0, 2), (1, 3)))
            V.tensor_add(out=apx(tA, 0, (1, 6 * nf)),
                         in0=apx(tA, 0, (1, 6 * nf)),
                         in1=apx(tB, 0, (1, 6 * nf)))
            V.tensor_mul(out=apx(tB, 0, (6, nf), (3, 2), (1, 3)),
                         in0=apx(X, pb + 2, (9, nf), (3, 2), (0, 3)),
                         in1=apx(X, cb + 6, (9, nf), (0, 2), (1, 3)))
            V.tensor_add(out=apx(X, cb, (9, nf), (3, 2), (1, 3)),
                         in0=apx(tA, 0, (6, nf), (3, 2), (1, 3)),
                         in1=apx(tB, 0, (6, nf), (3, 2), (1, 3)))

    def emit_w(V, X, w, dof, tA, tB, T, F, f0, nf, fw):
        """w[t, f, c] = d * col0(R_inblock) for lanes [f0, f0+nf); R20 via
        cross product kept in tA (local lane index f-f0, row width fw)."""
        d_ap = apx(dof, f0 * T * 4 + 2, (T * 4, nf), (4, T))
        V.tensor_mul(out=apx(tA, 0, (fw, T), (1, nf)),
                     in0=apx(X, f0 * 9 + 1, (F * 9, T), (9, nf)),
                     in1=apx(X, f0 * 9 + 5, (F * 9, T), (9, nf)))
        V.tensor_mul(out=apx(tB, 0, (fw, T), (1, nf)),
                     in0=apx(X, f0 * 9 + 2, (F * 9, T), (9, nf)),
                     in1=apx(X, f0 * 9 + 4, (F * 9, T), (9, nf)))
        V.tensor_sub(out=apx(tA, 0, (fw, T), (1, nf)),
                     in0=apx(tA, 0, (fw, T), (1, nf)),
                     in1=apx(tB, 0, (fw, T), (1, nf)))
        V.tensor_mul(out=apx(w, f0 * 3 + 2, (3, nf), (F * 3, T)),
                     in0=apx(tA, 0, (1, nf), (fw, T)),
                     in1=d_ap)
        V.tensor_mul(out=apx(w, f0 * 3 + 0, (3, nf), (F * 3, T)),
                     in0=apx(X, f0 * 9 + 0, (9, nf), (F * 9, T)),
                     in1=d_ap)
        V.tensor_mul(out=apx(w, f0 * 3 + 1, (3, nf), (F * 3, T)),
                     in0=apx(X, f0 * 9 + 3, (9, nf), (F * 9, T)),
                     in1=d_ap)

    def emit_cumsum(V, w, T, F, f0, nf):
        for t in range(1, T):
            V.tensor_add(out=apx(w, t * F * 3 + f0 * 3, (1, nf * 3)),
                         in0=apx(w, t * F * 3 + f0 * 3, (1, nf * 3)),
                         in1=apx(w, (t - 1) * F * 3 + f0 * 3, (1, nf * 3)))

    def emit_bht(V, X, w, bht, tA, tB, T, F, f0, nf, fw):
        """Assemble 12-elem (3x4 row-major) block-total HTs from the scan
        state at slab T-1 (+ row2 cross products; R20 reused from tA)."""
        base = (T - 1) * F * 9 + f0 * 9
        V.tensor_copy(out=apx(bht, f0 * 12, (12, nf), (4, 2), (1, 3)),
                      in_=apx(X, base, (9, nf), (3, 2), (1, 3)))
        V.tensor_copy(out=apx(bht, f0 * 12 + 8, (12, nf)),
                      in_=apx(tA, (T - 1) * fw, (1, nf)))
        # r21 = r02*r10 - r00*r12 ; r22 = r00*r11 - r01*r10
        for dst, (i1, i2), (i3, i4) in ((9, (2, 3), (0, 5)),
                                        (10, (0, 4), (1, 3))):
            V.tensor_mul(out=apx(tA, 0, (1, nf)),
                         in0=apx(X, base + i1, (9, nf)),
                         in1=apx(X, base + i2, (9, nf)))
            V.tensor_mul(out=apx(tB, 0, (1, nf)),
                         in0=apx(X, base + i3, (9, nf)),
                         in1=apx(X, base + i4, (9, nf)))
            V.tensor_sub(out=apx(bht, f0 * 12 + dst, (12, nf)),
                         in0=apx(tA, 0, (1, nf)),
                         in1=apx(tB, 0, (1, nf)))
        V.tensor_copy(out=apx(bht, f0 * 12 + 3, (12, nf), (4, 3)),
                      in_=apx(w, (T - 1) * F * 3 + f0 * 3, (3, nf), (1, 3)))

    def emit_levels(V, SC, bht, lp2, spx, rx, tA, tB, S, U, seed_rbr=None):
        """level2 (supers), level3 (exclusive over supers), excl_blocks."""
        CS = CHI * S
        LPS = (U + 1) * 12
        V.memset(lp2[:], 0.0)
        V.memset(apx(lp2, 0, (LPS, CS), (5, 3)), 1.0)
        SC.copy(out=apx(lp2, 12, (LPS, CS), (1, 12)),
                in_=apx(bht, 0, (U * 12, CS), (1, 12)))
        for u in range(1, U):
            compose_1d(V, CS,
                       a_off=u * 12, a_step=LPS,
                       b_off=u * 12, b_step=U * 12,
                       o_off=(u + 1) * 12, o_step=LPS,
                       tA=tA, tB=tB, a_tile=lp2, b_tile=bht, o_tile=lp2)
        if seed_rbr is None:
            V.memset(spx[:], 0.0)
            V.memset(apx(spx, 0, (S * 12, CHI), (5, 3)), 1.0)
        else:
            V.tensor_copy(out=apx(spx, 0, (S * 12, CHI), (1, 12)),
                          in_=apx(seed_rbr, 0, (12, CHI), (1, 12)))
        for s in range(1, S):
            compose_1d(V, CHI,
                       a_off=(s - 1) * 12, a_step=S * 12,
                       b_off=(s - 1) * LPS + U * 12, b_step=S * LPS,
                       o_off=s * 12, o_step=S * 12,
                       tA=tA, tB=tB, a_tile=spx, b_tile=lp2, o_tile=spx)
        excl_blocks(V, CS, U, LPS, spx, lp2, rx, tA, tB)

    def emit_down(V, w, rx, xyz, tA, tB, T, F, f0, nf):
        """xyz[f, t, i] = (R_bexcl @ w_cum)[i] + t_bexcl[i]."""
        for i in range(3):
            V.tensor_mul(out=apx(tA, 0, (T, nf), (1, T)),
                         in0=apx(rx, f0 * 12 + 4 * i + 0, (12, nf), (0, T)),
                         in1=apx(w, f0 * 3 + 0, (3, nf), (F * 3, T)))
            V.tensor_mul(out=apx(tB, 0, (T, nf), (1, T)),
                         in0=apx(rx, f0 * 12 + 4 * i + 1, (12, nf), (0, T)),
                         in1=apx(w, f0 * 3 + 1, (3, nf), (F * 3, T)))
            V.tensor_add(out=apx(tA, 0, (1, nf * T)),
                         in0=apx(tA, 0, (1, nf * T)),
                         in1=apx(tB, 0, (1, nf * T)))
            V.tensor_mul(out=apx(tB, 0, (T, nf), (1, T)),
                         in0=apx(rx, f0 * 12 + 4 * i + 2, (12, nf), (0, T)),
                         in1=apx(w, f0 * 3 + 2, (3, nf), (F * 3, T)))
            V.tensor_add(out=apx(tB, 0, (T, nf), (1, T)),
                         in0=apx(tB, 0, (T, nf), (1, T)),
                         in1=apx(rx, f0 * 12 + 4 * i + 3, (12, nf), (0, T)))
            V.tensor_add(out=apx(xyz, f0 * T * 3 + i, (T * 3, nf), (3, T)),
                         in0=apx(tA, 0, (T, nf), (1, T)),
                         in1=apx(tB, 0, (T, nf), (1, T)))

    FS0 = 84          # gen0 lane split: DVE [0,FS0), Pool [FS0,F0)
    DS0 = 52          # gen0 down-transform split
    TPW = 1024        # pool-engine temp width

    with tile.TileContext(nc) as tc:
      for _rep in range(repeat):
        with tc.tile_pool(name="main", bufs=1) as mp:
            X0 = mp.tile([P, T0 * F0 * 9], f32)
            dof0 = mp.tile([P, CHI * L0 * 4], f32)
            w0 = mp.tile([P, T0 * F0 * 3], f32)
            tA0 = mp.tile([P, max(T0 * F0, F0 * 12)], f32)
            tB0 = mp.tile([P, max(T0 * F0, F0 * 12)], f32)
            tAp = mp.tile([P, TPW], f32)
            tBp = mp.tile([P, TPW], f32)
            rx0 = mp.tile([P, F0 * 12], f32)
            rbr = mp.tile([P, CHI * 12], f32)
            a32 = mp.tile([P, CHI * 12], f32)
            jd = mp.tile([P, CHI * 9], f32)
            jang = mp.tile([P, CHI * 2 * 3], f32)
            jsin = mp.tile([P, CHI * 2 * 3], f32)
            jcos = mp.tile([P, CHI * 2 * 3], f32)
            re_ = mp.tile([P, CHI * 2 * 9], f32)
            rj = mp.tile([P, CHI * 9], f32)
            jtmp = mp.tile([P, CHI * 2 * 9], f32)
            halfpi = mp.tile([P, 1], f32)

            nc.sync.dma_start(out=jd[:], in_=jd_d[:])
            nc.vector.memset(halfpi[:], PI / 2)

            V = nc.vector
            G = nc.gpsimd
            SC = nc.scalar
            stt = V.scalar_tensor_tensor

            src = AP(b0_d, 0, [[L0 * 4, P], [P * L0 * 4, CHI], [1, L0 * 4]])
            dst = AP(dof0[:].tensor, dof0[:].offset,
                     [[dof0[:].ap[0][0], P], [L0 * 4, CHI], [1, L0 * 4]])
            nc.sync.dma_start(out=dst, in_=src)

            # ---- gen0 front (DVE + ACT) ----
            with tc.tile_pool(name="ptrig0", bufs=1) as pt:
                trig = {nm: pt.tile([P, CHI * L0], f32, name=f"t0_{nm}")
                        for nm in ("cp", "sp", "ct", "st", "cc", "sc",
                                   "tm1", "tm2", "tu", "tv")}
                aw = pt.tile([P, CHI * L0], f32)
                emit_trig(V, SC, dof0, trig, aw, L0, halfpi)
                emit_bond(V, stt, trig, X0, T0, F0, 0, F0)

            with tc.tile_pool(name="prest", bufs=1) as pr:
                X1 = pr.tile([P, T1 * F1 * 9], f32)
                dof1 = pr.tile([P, CHI * L1 * 4], f32)

                src = AP(b1_d, 0, [[L1 * 4, P], [P * L1 * 4, CHI], [1, L1 * 4]])
                dst = AP(dof1[:].tensor, dof1[:].offset,
                         [[dof1[:].ap[0][0], P], [L1 * 4, CHI], [1, L1 * 4]])
                nc.sync.dma_start(out=dst, in_=src)

                # ---- gen1 front (DVE + ACT) ----
                with tc.tile_pool(name="ptrig1", bufs=1) as pt1:
                    trig1 = {nm: pt1.tile([P, CHI * L1], f32, name=f"t1_{nm}")
                             for nm in ("cp", "sp", "ct", "st", "cc", "sc",
                                        "tm1", "tm2", "tu", "tv")}
                    aw1 = pt1.tile([P, CHI * L1], f32)
                    emit_trig(V, SC, dof1, trig1, aw1, L1, halfpi)
                    emit_bond(V, stt, trig1, X1, T1, F1, 0, F1)

                with tc.tile_pool(name="pwork", bufs=1) as pw:
                    bht0 = pw.tile([P, F0 * 12], f32)
                    lp2_0 = pw.tile([P, CHI * S0 * (U0 + 1) * 12], f32)
                    spx0 = pw.tile([P, CHI * S0 * 12], f32)
                    w1 = pw.tile([P, T1 * F1 * 3], f32)
                    bht1 = pw.tile([P, F1 * 12], f32)
                    lp2_1 = pw.tile([P, CHI * S1 * (U1 + 1) * 12], f32)
                    spx1 = pw.tile([P, CHI * S1 * 12], f32)
                    rx1 = pw.tile([P, F1 * 12], f32)

                    # ---- JUMP HT rotation for chain-start lanes (DVE) ----
                    V.tensor_copy(out=jang[:], in_=apx(jd, 3, (9, CHI), (3, 2),
                                                       (1, 3)))
                    V.add_range_wrap(out=jsin[:], in_=jang[:], shift=0.0,
                                     bound=PI, period=2 * PI)
                    SC.activation(out=jsin[:], in_=jsin[:], func=SIN)
                    V.add_range_wrap(out=jcos[:], in_=jang[:], shift=PI / 2,
                                     bound=PI, period=2 * PI)
                    SC.activation(out=jcos[:], in_=jcos[:], func=SIN)

                    CR = CHI * 2

                    def sc_(tl, ang):
                        return apx(tl, ang, (3, CR))

                    def re(e):
                        return apx(re_, e, (9, CR))

                    def jt1(e):
                        return apx(jtmp, e, (9, CR))

                    sa = lambda: sc_(jsin, 0)
                    sb = lambda: sc_(jsin, 1)
                    s_c = lambda: sc_(jsin, 2)
                    ca = lambda: sc_(jcos, 0)
                    cb = lambda: sc_(jcos, 1)
                    c_c = lambda: sc_(jcos, 2)
                    # R = Rz(c)Ry(b)Rx(a) per (chi, rot) lane
                    V.tensor_mul(out=re(0), in0=c_c(), in1=cb())
                    V.tensor_mul(out=jt1(0), in0=sb(), in1=sa())
                    V.tensor_mul(out=jt1(1), in0=sb(), in1=ca())
                    V.tensor_mul(out=jt1(2), in0=c_c(), in1=jt1(0))
                    V.tensor_mul(out=jt1(3), in0=s_c(), in1=ca())
                    V.tensor_sub(out=re(1), in0=jt1(2), in1=jt1(3))
                    V.tensor_mul(out=jt1(2), in0=c_c(), in1=jt1(1))
                    V.tensor_mul(out=jt1(3), in0=s_c(), in1=sa())
                    V.tensor_add(out=re(2), in0=jt1(2), in1=jt1(3))
                    V.tensor_mul(out=re(3), in0=s_c(), in1=cb())
                    V.tensor_mul(out=jt1(2), in0=s_c(), in1=jt1(0))
                    V.tensor_mul(out=jt1(3), in0=c_c(), in1=ca())
                    V.tensor_add(out=re(4), in0=jt1(2), in1=jt1(3))
                    V.tensor_mul(out=jt1(2), in0=s_c(), in1=jt1(1))
                    V.tensor_mul(out=jt1(3), in0=c_c(), in1=sa())
                    V.tensor_sub(out=re(5), in0=jt1(2), in1=jt1(3))
                    V.tensor_scalar_mul(out=re(6), in0=sb(), scalar1=-1.0)
                    V.tensor_mul(out=re(7), in0=cb(), in1=sa())
                    V.tensor_mul(out=re(8), in0=cb(), in1=ca())
                    # rj = R1 @ R2 (3x3), lanes = chi
                    V.tensor_mul(
                        out=apx(rj, 0, (9, CHI), (3, 3), (1, 3)),
                        in0=apx(re_, 0, (18, CHI), (3, 3), (0, 3)),
                        in1=apx(re_, 9, (18, CHI), (0, 3), (1, 3)))
                    V.tensor_mul(
                        out=apx(jtmp, 0, (9, CHI), (3, 3), (1, 3)),
                        in0=apx(re_, 1, (18, CHI), (3, 3), (0, 3)),
                        in1=apx(re_, 12, (18, CHI), (0, 3), (1, 3)))
                    V.tensor_add(out=rj[:, : CHI * 9], in0=rj[:, : CHI * 9],
                                 in1=jtmp[:, : CHI * 9])
                    V.tensor_mul(
                        out=apx(jtmp, 0, (9, CHI), (3, 3), (1, 3)),
                        in0=apx(re_, 2, (18, CHI), (3, 3), (0, 3)),
                        in1=apx(re_, 15, (18, CHI), (0, 3), (1, 3)))
                    V.tensor_add(out=rj[:, : CHI * 9], in0=rj[:, : CHI * 9],
                                 in1=jtmp[:, : CHI * 9])
                    # full jump 3x3 -> X0 slab 0, lane f=chi*J0 (j=0)
                    V.tensor_copy(out=apx(X0, 0, (J0 * 9, CHI), (1, 9)),
                                  in_=apx(rj, 0, (9, CHI), (1, 9)))

                    # ---- gen0 level-1: DVE [0,FS0) || Pool [FS0,F0) ----
                    emit_scan(V, X0, tA0, tB0, T0, F0, 0, FS0)
                    emit_scan(G, X0, tAp, tBp, T0, F0, FS0, F0 - FS0)
                    emit_w(V, X0, w0, dof0, tA0, tB0, T0, F0, 0, FS0, F0)
                    # jump translation overwrites w at (t=0, j=0) lanes
                    V.tensor_copy(out=apx(w0, 0, (J0 * 3, CHI), (1, 3)),
                                  in_=apx(jd, 0, (9, CHI), (1, 3)))
                    emit_cumsum(V, w0, T0, F0, 0, FS0)
                    emit_bht(V, X0, w0, bht0, tA0, tB0, T0, F0, 0, FS0, F0)
                    emit_w(G, X0, w0, dof0, tAp, tBp, T0, F0, FS0,
                           F0 - FS0, F0 - FS0)
                    emit_cumsum(G, w0, T0, F0, FS0, F0 - FS0)
                    emit_bht(G, X0, w0, bht0, tAp, tBp, T0, F0, FS0,
                             F0 - FS0, F0 - FS0)

                    # ---- gen0 block levels + branch roots (DVE) ----
                    emit_levels(V, SC, bht0, lp2_0, spx0, rx0, tA0, tB0,
                                S0, U0)
                    V.tensor_copy(out=apx(a32, 0, (12, CHI), (4, 2), (1, 3)),
                                  in_=apx(X0, 32 * 9, (J0 * 9, CHI), (3, 2),
                                          (1, 3)))
                    for dsti, (i1, i2), (i3, i4) in ((8, (1, 5), (2, 4)),
                                                     (9, (2, 3), (0, 5)),
                                                     (10, (0, 4), (1, 3))):
                        V.tensor_mul(out=apx(tA0, 0, (1, CHI)),
                                     in0=apx(X0, 32 * 9 + i1, (J0 * 9, CHI)),
                                     in1=apx(X0, 32 * 9 + i2, (J0 * 9, CHI)))
                        V.tensor_mul(out=apx(tB0, 0, (1, CHI)),
                                     in0=apx(X0, 32 * 9 + i3, (J0 * 9, CHI)),
                                     in1=apx(X0, 32 * 9 + i4, (J0 * 9, CHI)))
                        V.tensor_sub(out=apx(a32, dsti, (12, CHI)),
                                     in0=apx(tA0, 0, (1, CHI)),
                                     in1=apx(tB0, 0, (1, CHI)))
                    V.tensor_copy(out=apx(a32, 3, (12, CHI), (4, 3)),
                                  in_=apx(w0, 32 * 3, (J0 * 3, CHI), (1, 3)))
                    compose_1d(V, CHI,
                               a_off=32 * 12, a_step=J0 * 12,
                               b_off=0, b_step=12,
                               o_off=0, o_step=12,
                               tA=tA0, tB=tB0,
                               a_tile=rx0, b_tile=a32, o_tile=rbr)

                    # ---- gen1 level-1 (Pool) ----
                    emit_scan(G, X1, tAp, tBp, T1, F1, 0, F1)
                    emit_w(G, X1, w1, dof1, tAp, tBp, T1, F1, 0, F1, F1)
                    emit_cumsum(G, w1, T1, F1, 0, F1)
                    emit_bht(G, X1, w1, bht1, tAp, tBp, T1, F1, 0, F1, F1)

                    # ---- gen1 levels (DVE, seeded by rbr) ----
                    emit_levels(V, SC, bht1, lp2_1, spx1, rx1, tA0, tB0,
                                S1, U1, seed_rbr=rbr)

                    # ---- final transforms; xyz aliases dead X scan space ----
                    emit_down(V, w0, rx0, X0, tA0, tB0, T0, F0, 0, DS0)
                    nc.sync.dma_start(
                        out=AP(kin0_d, 0, [[F0 * T0 * 3, P], [1, DS0 * T0 * 3]]),
                        in_=apx(X0, 0, (1, DS0 * T0 * 3)))
                    emit_down(G, w0, rx0, X0, tAp, tBp, T0, F0, DS0, F0 - DS0)
                    nc.sync.dma_start(
                        out=AP(kin0_d, DS0 * T0 * 3,
                               [[F0 * T0 * 3, P], [1, (F0 - DS0) * T0 * 3]]),
                        in_=apx(X0, DS0 * T0 * 3, (1, (F0 - DS0) * T0 * 3)))
                    emit_down(V, w1, rx1, X1, tA0, tB0, T1, F1, 0, F1)
                    nc.sync.dma_start(
                        out=AP(kin1_d, 0, [[F1 * T1 * 3, P], [1, F1 * T1 * 3]]),
                        in_=apx(X1, 0, (1, F1 * T1 * 3)))

    nc.compile()
    return nc


def get_program(repeat=1):
    key = ("nc", repeat)
    if key not in _CACHE:
        _CACHE[key] = _build_program(repeat)
    return _CACHE[key]


# ------------------------------------------------------------------- host
def _shard_inputs(dofs, doftype):
    """Build the 8 per-core input maps (lane order (p, chi, j, t))."""
    in_maps = []
    chain_starts = 1 + np.arange(C0, dtype=np.int64) * L0
    jd_all = np.ascontiguousarray(dofs[chain_starts])       # [C0, 9]
    for core in range(NCORES):
        g0 = np.ascontiguousarray(
            dofs[1 + core * A0: 1 + (core + 1) * A0, :4])
        g1 = np.ascontiguousarray(
            dofs[BOFF + core * A1: BOFF + (core + 1) * A1, :4])
        jd = np.ascontiguousarray(
            jd_all[core * CH0:(core + 1) * CH0]
            .reshape(CHI, P, 9).transpose(1, 0, 2).reshape(P, CHI * 9))
        in_maps.append({"b0": g0, "b1": g1, "jd": jd})
    return in_maps


def _lane_ids(id_idx, core):
    """id_idx values of this core's atoms in device lane order (p, f, t)."""
    ids0 = (id_idx[core * A0:(core + 1) * A0]
            .reshape(CHI, P, L0).transpose(1, 0, 2).ravel())
    ids1 = (id_idx[BOFF - 1 + core * A1: BOFF - 1 + (core + 1) * A1]
            .reshape(CHI, P, L1).transpose(1, 0, 2).ravel())
    return ids0, ids1


def _structure_ok(doftype, gen0_paths, gen1_paths):
    chain_starts = 1 + np.arange(C0, dtype=np.int64) * L0
    g0 = np.concatenate(
        [np.zeros((C0, 1), np.int64), chain_starts[:, None] + np.arange(L0)],
        axis=1)
    if not np.array_equal(gen0_paths, g0.astype(gen0_paths.dtype)):
        return False
    branch_roots = chain_starts + L0 // 2
    g1 = np.concatenate(
        [branch_roots[:, None],
         BOFF + (np.arange(C1, dtype=np.int64) * L1)[:, None] + np.arange(L1)],
        axis=1)
    if not np.array_equal(gen1_paths, g1.astype(gen1_paths.dtype)):
        return False
    if doftype[0] != 0:
        return False
    if not np.all(doftype[chain_starts] == 1):
        return False
    dt = doftype.copy()
    dt[chain_starts] = 2
    if not np.all(dt[1:] == 2):
        return False
    return True


def _numpy_fallback(dofs, doftype, gen0_paths, gen1_paths, id_idx):
    """Exact numpy port of the reference (slow path, safety net)."""
    def rx(a):
        c, s = np.cos(a), np.sin(a)
        o, z = np.ones_like(a), np.zeros_like(a)
        return np.stack([np.stack([o, z, z, z], -1), np.stack([z, c, -s, z], -1),
                         np.stack([z, s, c, z], -1), np.stack([z, z, z, o], -1)], -2)

    def ry(a):
        c, s = np.cos(a), np.sin(a)
        o, z = np.ones_like(a), np.zeros_like(a)
        return np.stack([np.stack([c, z, s, z], -1), np.stack([z, o, z, z], -1),
                         np.stack([-s, z, c, z], -1), np.stack([z, z, z, o], -1)], -2)

    def rz(a):
        c, s = np.cos(a), np.sin(a)
        o, z = np.ones_like(a), np.zeros_like(a)
        return np.stack([np.stack([c, -s, z, z], -1), np.stack([s, c, z, z], -1),
                         np.stack([z, z, o, z], -1), np.stack([z, z, z, o], -1)], -2)

    def trans(x, y, z):
        o, zr = np.ones_like(x), np.zeros_like(x)
        return np.stack([np.stack([o, zr, zr, x], -1), np.stack([zr, o, zr, y], -1),
                         np.stack([zr, zr, o, z], -1), np.stack([zr, zr, zr, o], -1)], -2)

    dofs = dofs.astype(np.float32)
    phi_p, theta, d, phi_c = dofs[:, 0], dofs[:, 1], dofs[:, 2], dofs[:, 3]
    z = np.zeros_like(d)
    bond = rx(phi_p) @ rz(np.pi - theta) @ trans(d, z, z) @ rx(phi_c)
    rot = lambda a, b, c: rz(c) @ ry(b) @ rx(a)
    jump = (trans(dofs[:, 0], dofs[:, 1], dofs[:, 2])
            @ rot(dofs[:, 3], dofs[:, 4], dofs[:, 5])
            @ rot(dofs[:, 6], dofs[:, 7], dofs[:, 8]))
    eye = np.broadcast_to(np.eye(4, dtype=dofs.dtype), bond.shape)
    dt = doftype[:, None, None]
    hts = np.where(dt == 1, jump, np.where(dt == 2, bond, eye)).astype(np.float32)
    for paths in (gen0_paths, gen1_paths):
        seg = hts[paths]
        out = np.empty_like(seg)
        out[:, 0] = seg[:, 0]
        for i in range(1, seg.shape[1]):
            out[:, i] = out[:, i - 1] @ seg[:, i]
        hts[paths] = out
    kincoords = hts[:, :3, 3]
    coords = np.zeros((N - 1, 3), dtype=dofs.dtype)
    coords[np.asarray(id_idx)] = kincoords[1:]
    return coords


def kernel(dofs, doftype, gen0_paths, gen1_paths, id_idx):
    dofs = np.asarray(dofs, dtype=np.float32)
    doftype = np.asarray(doftype, dtype=np.int32)
    gen0_paths = np.asarray(gen0_paths)
    gen1_paths = np.asarray(gen1_paths)
    id_idx = np.asarray(id_idx, dtype=np.int32)

    if not _structure_ok(doftype, gen0_paths, gen1_paths):
        return _numpy_fallback(dofs, doftype, gen0_paths, gen1_paths, id_idx)

    from concourse.bass_utils import run_bass_kernel_spmd

    nc = get_program()
    in_maps = _shard_inputs(dofs, doftype)
    res = run_bass_kernel_spmd(nc, in_maps, core_ids=list(range(NCORES)))
    out = np.empty((N - 1, 3), dtype=np.float32)
    for core in range(NCORES):
        ids0, ids1 = _lane_ids(id_idx, core)
        out[ids0] = res.results[core]["kin0"].reshape(-1, 3)
        out[ids1] = res.results[core]["kin1"].reshape(-1, 3)
    return out


# revision 8
# speedup vs baseline: 1.2570x; 1.2570x over previous
"""Trainium2 Bass kernel for nn_KinematicOperation (kinematic tree forward).

Structure of the (deterministic) problem instance:
  - N = 1 + 2048*768 + 2048*256 atoms.
  - gen0: 2048 chains of 768 atoms rooted at the virtual root (identity HT);
    chain atoms are contiguous: chain c = atoms [1+c*768, 1+(c+1)*768).
  - gen1: 2048 branches of 256 atoms rooted mid-chain (gen0 chain c position
    384); branch atoms contiguous starting at boff = 1 + 2048*768.
  - Local HTs: BOND everywhere except a JUMP at each chain start; root = I.
  - Output: coords[id_idx[a-1]] = prefix_HT(a)[:3, 3] for atoms a = 1..N-1.

Sharding: core k owns gen0 chains [256k, 256(k+1)) and gen1 branches of the
same index range, so the branch-root HT handoff between generations stays
on-core and no collectives are needed.  Host pre-slices bond dof columns
(0..3 of 9) and gathers jump rows, shrinking input DMA.

Device algorithm per generation (fp32; rotations stored as 3x3 row-major,
translations separately):
  - ACT computes sin/cos (one DVE range-wrap per angle; cos as
    sin(pi/2 - |w|)); DVE assembles the local 3x3 rotations into SBUF.
  - 3-level blocked prefix scan along each chain:
      level1: rotation-only scan propagating ROWS 0,1 (6 elems) in place;
      translations via the NeRF identity local_t = d * col0(localR):
      t_glob(p) = sum_{q<=p} d_q * col0(R_glob_q), so in-block translations
      are prefix SUMS of w = d * col0(R_inblock) (col0 z-comp from a cross
      product), then level2/3 compose full 3x4 block HTs (tiny), and the
      final transform applies block-exclusive R,t to the in-block cumsums.
  - Output xyz written scatter-ready; host applies the id_idx permutation.
"""

import os
import sys

import numpy as np

for _p in ("/opt/trn_rl_repo", "/root/.axon_site/_ro/trn_rl_repo"):
    if os.path.isdir(_p) and _p not in sys.path:
        sys.path.insert(0, _p)

# ---------------------------------------------------------------- constants
C0, L0 = 2048, 768
C1, L1 = 2048, 256
N = 1 + C0 * L0 + C1 * L1
BOFF = 1 + C0 * L0
NCORES = 8
P = 128
CHI = 2                      # chains per partition (256 chains per core)
CH0 = C0 // NCORES
CH1 = C1 // NCORES
A0 = CH0 * L0                # 196608 gen0 atoms per core
A1 = CH1 * L1                # 65536 gen1 atoms per core

# block geometry: L = T*J,  J = S*U supers x blocks
T0, J0, S0, U0 = 12, 64, 8, 8
F0 = CHI * J0                # 128 block-lanes per partition
T1, J1, S1, U1 = 8, 32, 4, 8
F1 = CHI * J1                # 64

PI = float(np.pi)

_CACHE = {}


# ------------------------------------------------------------- device build
def _build_program(repeat=1):
    from concourse import bacc, mybir, tile
    from concourse.bass import AP

    f32 = mybir.dt.float32
    MUL = mybir.AluOpType.mult
    SUB = mybir.AluOpType.subtract
    SIN = mybir.ActivationFunctionType.Sin
    ABS = mybir.ActivationFunctionType.Abs

    nc = bacc.Bacc("TRN2", target_bir_lowering=False, debug=False)

    b0_d = nc.dram_tensor("b0", [A0, 4], f32, kind="ExternalInput")
    b1_d = nc.dram_tensor("b1", [A1, 4], f32, kind="ExternalInput")
    jd_d = nc.dram_tensor("jd", [P, CHI * 9], f32, kind="ExternalInput")
    kin0_d = nc.dram_tensor("kin0", [P, F0 * T0 * 3], f32, kind="ExternalOutput")
    kin1_d = nc.dram_tensor("kin1", [P, F1 * T1 * 3], f32, kind="ExternalOutput")

    def apx(tl, off, *dims):
        """AP over tile-AP `tl` at free-elem offset `off` with free dims
        [(step, count), ...] (full 128 partitions)."""
        t = tl[:] if not isinstance(tl, AP) else tl
        return AP(t.tensor, t.offset + off, [[t.ap[0][0], P]] + [list(d) for d in dims])

    def compose_1d(vec, lanes, a_off, a_step, b_off, b_step, o_off, o_step,
                   tA, tB, a_tile, b_tile, o_tile):
        """C = A @ B (3x4 HT compose, 12-elem row-major layout) over lanes."""
        for k, dst in ((0, tA), (1, tB)):
            vec.tensor_mul(
                out=apx(dst, 0, (12, lanes), (4, 3), (1, 4)),
                in0=apx(a_tile, a_off + k, (a_step, lanes), (4, 3), (0, 4)),
                in1=apx(b_tile, b_off + 4 * k, (b_step, lanes), (0, 3), (1, 4)),
            )
        vec.tensor_add(
            out=apx(tA, 0, (12, lanes), (1, 12)),
            in0=apx(tA, 0, (12, lanes), (1, 12)),
            in1=apx(tB, 0, (12, lanes), (1, 12)))
        vec.tensor_mul(
            out=apx(tB, 0, (12, lanes), (4, 3), (1, 4)),
            in0=apx(a_tile, a_off + 2, (a_step, lanes), (4, 3), (0, 4)),
            in1=apx(b_tile, b_off + 8, (b_step, lanes), (0, 3), (1, 4)),
        )
        vec.tensor_add(
            out=apx(o_tile, o_off, (o_step, lanes), (1, 12)),
            in0=apx(tA, 0, (12, lanes), (1, 12)),
            in1=apx(tB, 0, (12, lanes), (1, 12)),
        )
        vec.tensor_add(
            out=apx(o_tile, o_off + 3, (o_step, lanes), (4, 3)),
            in0=apx(o_tile, o_off + 3, (o_step, lanes), (4, 3)),
            in1=apx(a_tile, a_off + 3, (a_step, lanes), (4, 3)),
        )

    def excl_blocks(vec, CS, U, LPS, spx, lp2, rx, tA, tB):
        """rx[cs, u] = spx[cs] @ lp2[cs, u]  (exclusive block prefixes)."""
        for i in range(3):
            for k, dst in ((0, tA), (1, tB)):
                vec.tensor_mul(
                    out=apx(dst, 4 * i, (96, CS), (12, U), (1, 4)),
                    in0=apx(spx, 4 * i + k, (12, CS), (0, U), (0, 4)),
                    in1=apx(lp2, 4 * k, (LPS, CS), (12, U), (1, 4)))
            vec.tensor_add(
                out=apx(tA, 4 * i, (96, CS), (12, U), (1, 4)),
                in0=apx(tA, 4 * i, (96, CS), (12, U), (1, 4)),
                in1=apx(tB, 4 * i, (96, CS), (12, U), (1, 4)))
            vec.tensor_mul(
                out=apx(tB, 4 * i, (96, CS), (12, U), (1, 4)),
                in0=apx(spx, 4 * i + 2, (12, CS), (0, U), (0, 4)),
                in1=apx(lp2, 8, (LPS, CS), (12, U), (1, 4)))
            vec.tensor_add(
                out=apx(rx, 4 * i, (96, CS), (12, U), (1, 4)),
                in0=apx(tA, 4 * i, (96, CS), (12, U), (1, 4)),
                in1=apx(tB, 4 * i, (96, CS), (12, U), (1, 4)))
        vec.tensor_add(
            out=apx(rx, 3, (96, CS), (12, U), (4, 3)),
            in0=apx(rx, 3, (96, CS), (12, U), (4, 3)),
            in1=apx(spx, 3, (12, CS), (0, U), (4, 3)))

    # ---- generation emitters (engine-parameterized, f-lane-ranged) ----
    # Lane f = chi*J + j indexes a block; chain position = j*T + t, and the
    # per-atom source index chi*L + j*T + t == f*T + t, so every per-atom
    # phase uses 2-D (f, t) access patterns and splits at any f boundary.

    def emit_trig_fold(V, S, dof, trig, L, halfpi, alpha_fix):
        """Angle-folded trig: alpha_p = phi_c[p-1] + phi_p[p]; planes
        sa/ca = sin/cos(alpha), st/ct = sin/cos(theta).  One DVE wrap per
        angle (cos plane as scratch), cos = sin(pi/2 - |w|) on ACT.
        alpha_fix(apl) patches the chain-start alpha values."""
        apl, aw = trig["apl"], trig["aw"]
        V.tensor_add(out=apx(apl, 1, (L, CHI), (1, L - 1)),
                     in0=apx(dof, 4, (L * 4, CHI), (4, L - 1)),
                     in1=apx(dof, 3, (L * 4, CHI), (4, L - 1)))
        alpha_fix(apl)
        for src, cosn, sinn in ((apx(apl, 0, (L, CHI), (1, L)), "ca", "sa"),
                                (apx(dof, 1, (L * 4, CHI), (4, L)),
                                 "ct", "st")):
            V.add_range_wrap(out=trig[cosn][:], in_=src, shift=0.0,
                             bound=PI, period=2 * PI)
            S.activation(out=trig[sinn][:], in_=trig[cosn][:], func=SIN)
            S.activation(out=aw[:], in_=trig[cosn][:], func=ABS)
            S.activation(out=trig[cosn][:], in_=aw[:], func=SIN,
                         scale=-1.0, bias=halfpi[:])

    def emit_init0(V, trig, X, T, F, f0, nf):
        """Slab-0 init: local folded factor Rx(a)Rz(pi-theta) =
        [[-ct, -st, 0], [ca*st, -ca*ct, -sa], [sa*st, -sa*ct, ca]]."""
        def tp(nm):
            return apx(trig[nm], f0 * T, (T, nf))

        def xo(e):
            return apx(X, f0 * 9 + e, (9, nf))

        V.tensor_scalar_mul(out=xo(0), in0=tp("ct"), scalar1=-1.0)
        V.tensor_scalar_mul(out=xo(1), in0=tp("st"), scalar1=-1.0)
        V.memset(apx(X, f0 * 9 + 2, (9, nf)), 0.0)
        V.tensor_mul(out=xo(3), in0=tp("ca"), in1=tp("st"))
        V.tensor_mul(out=xo(4), in0=tp("ca"), in1=xo(0))
        V.tensor_scalar_mul(out=xo(5), in0=tp("sa"), scalar1=-1.0)
        V.tensor_mul(out=xo(6), in0=tp("sa"), in1=tp("st"))
        V.tensor_mul(out=xo(7), in0=tp("sa"), in1=xo(0))
        V.tensor_copy(out=xo(8), in_=tp("ca"))

    def emit_scan(V, stt, X, trig, tA, tB, tC, T, F, f0, nf):
        """In-place folded scan of rotation rows 0,1 for lanes [f0,f0+nf):
        S_t = S_{t-1} * Rx(alpha_t) * Rz(pi-theta_t), columns in place:
          c1' = c1*ca + c2*sa          (temp)
          c2_t = c2*ca - c1*sa
          c0_t = c1'*st - c0*ct
          c1_t = -(c0*st + c1'*ct)
        """
        for t in range(1, T):
            pb = (t - 1) * F * 9 + f0 * 9
            cb = t * F * 9 + f0 * 9

            def col(e):
                return apx(X, pb + e, (9, nf), (3, 2))

            def colw(e):
                return apx(X, cb + e, (9, nf), (3, 2))

            def tp(nm):
                return apx(trig[nm], f0 * T + t, (T, nf), (0, 2))

            tAa = apx(tA, 0, (2, nf), (1, 2))
            tBa = apx(tB, 0, (2, nf), (1, 2))
            tCa = apx(tC, 0, (2, nf), (1, 2))
            V.tensor_mul(out=tAa, in0=col(1), in1=tp("ca"))
            V.tensor_mul(out=tBa, in0=col(2), in1=tp("sa"))
            V.tensor_add(out=apx(tC, 0, (1, 2 * nf)),
                         in0=apx(tA, 0, (1, 2 * nf)),
                         in1=apx(tB, 0, (1, 2 * nf)))
            V.tensor_mul(out=tAa, in0=col(1), in1=tp("sa"))
            V.tensor_mul(out=tBa, in0=col(2), in1=tp("ca"))
            V.tensor_sub(out=colw(2), in0=tBa, in1=tAa)
            V.tensor_mul(out=tAa, in0=tCa, in1=tp("st"))
            V.tensor_mul(out=tBa, in0=col(0), in1=tp("ct"))
            V.tensor_sub(out=colw(0), in0=tAa, in1=tBa)
            V.tensor_mul(out=tAa, in0=col(0), in1=tp("st"))
            V.tensor_mul(out=tBa, in0=tCa, in1=tp("ct"))
            stt(out=colw(1), in0=tAa, scalar=-1.0, in1=tBa,
                op0=MUL, op1=SUB)

    def emit_w(V, X, w, dof, tA, tB, T, F, f0, nf, fw):
        """w[t, f, c] = d * col0(R_inblock) for lanes [f0, f0+nf); R20 via
        cross product kept in tA (local lane index f-f0, row width fw)."""
        d_ap = apx(dof, f0 * T * 4 + 2, (T * 4, nf), (4, T))
        V.tensor_mul(out=apx(tA, 0, (fw, T), (1, nf)),
                     in0=apx(X, f0 * 9 + 1, (F * 9, T), (9, nf)),
                     in1=apx(X, f0 * 9 + 5, (F * 9, T), (9, nf)))
        V.tensor_mul(out=apx(tB, 0, (fw, T), (1, nf)),
                     in0=apx(X, f0 * 9 + 2, (F * 9, T), (9, nf)),
                     in1=apx(X, f0 * 9 + 4, (F * 9, T), (9, nf)))
        V.tensor_sub(out=apx(tA, 0, (fw, T), (1, nf)),
                     in0=apx(tA, 0, (fw, T), (1, nf)),
                     in1=apx(tB, 0, (fw, T), (1, nf)))
        V.tensor_mul(out=apx(w, f0 * 3 + 2, (3, nf), (F * 3, T)),
                     in0=apx(tA, 0, (1, nf), (fw, T)),
                     in1=d_ap)
        V.tensor_mul(out=apx(w, f0 * 3 + 0, (3, nf), (F * 3, T)),
                     in0=apx(X, f0 * 9 + 0, (9, nf), (F * 9, T)),
                     in1=d_ap)
        V.tensor_mul(out=apx(w, f0 * 3 + 1, (3, nf), (F * 3, T)),
                     in0=apx(X, f0 * 9 + 3, (9, nf), (F * 9, T)),
                     in1=d_ap)

    def emit_cumsum(V, w, T, F, f0, nf):
        for t in range(1, T):
            V.tensor_add(out=apx(w, t * F * 3 + f0 * 3, (1, nf * 3)),
                         in0=apx(w, t * F * 3 + f0 * 3, (1, nf * 3)),
                         in1=apx(w, (t - 1) * F * 3 + f0 * 3, (1, nf * 3)))

    def emit_bht(V, X, w, bht, tA, tB, T, F, f0, nf, fw):
        """Assemble 12-elem (3x4 row-major) block-total HTs from the scan
        state at slab T-1 (+ row2 cross products; R20 reused from tA)."""
        base = (T - 1) * F * 9 + f0 * 9
        V.tensor_copy(out=apx(bht, f0 * 12, (12, nf), (4, 2), (1, 3)),
                      in_=apx(X, base, (9, nf), (3, 2), (1, 3)))
        V.tensor_copy(out=apx(bht, f0 * 12 + 8, (12, nf)),
                      in_=apx(tA, (T - 1) * fw, (1, nf)))
        # r21 = r02*r10 - r00*r12 ; r22 = r00*r11 - r01*r10
        for dst, (i1, i2), (i3, i4) in ((9, (2, 3), (0, 5)),
                                        (10, (0, 4), (1, 3))):
            V.tensor_mul(out=apx(tA, 0, (1, nf)),
                         in0=apx(X, base + i1, (9, nf)),
                         in1=apx(X, base + i2, (9, nf)))
            V.tensor_mul(out=apx(tB, 0, (1, nf)),
                         in0=apx(X, base + i3, (9, nf)),
                         in1=apx(X, base + i4, (9, nf)))
            V.tensor_sub(out=apx(bht, f0 * 12 + dst, (12, nf)),
                         in0=apx(tA, 0, (1, nf)),
                         in1=apx(tB, 0, (1, nf)))
        V.tensor_copy(out=apx(bht, f0 * 12 + 3, (12, nf), (4, 3)),
                      in_=apx(w, (T - 1) * F * 3 + f0 * 3, (3, nf), (1, 3)))

    def emit_levels(V, SC, bht, lp2, spx, rx, tA, tB, S, U, seed_rbr=None):
        """level2 (supers), level3 (exclusive over supers), excl_blocks."""
        CS = CHI * S
        LPS = (U + 1) * 12
        V.memset(lp2[:], 0.0)
        V.memset(apx(lp2, 0, (LPS, CS), (5, 3)), 1.0)
        SC.copy(out=apx(lp2, 12, (LPS, CS), (1, 12)),
                in_=apx(bht, 0, (U * 12, CS), (1, 12)))
        for u in range(1, U):
            compose_1d(V, CS,
                       a_off=u * 12, a_step=LPS,
                       b_off=u * 12, b_step=U * 12,
                       o_off=(u + 1) * 12, o_step=LPS,
                       tA=tA, tB=tB, a_tile=lp2, b_tile=bht, o_tile=lp2)
        if seed_rbr is None:
            V.memset(spx[:], 0.0)
            V.memset(apx(spx, 0, (S * 12, CHI), (5, 3)), 1.0)
        else:
            V.tensor_copy(out=apx(spx, 0, (S * 12, CHI), (1, 12)),
                          in_=apx(seed_rbr, 0, (12, CHI), (1, 12)))
        for s in range(1, S):
            compose_1d(V, CHI,
                       a_off=(s - 1) * 12, a_step=S * 12,
                       b_off=(s - 1) * LPS + U * 12, b_step=S * LPS,
                       o_off=s * 12, o_step=S * 12,
                       tA=tA, tB=tB, a_tile=spx, b_tile=lp2, o_tile=spx)
        excl_blocks(V, CS, U, LPS, spx, lp2, rx, tA, tB)

    def emit_down(V, w, rx, xyz, tA, tB, T, F, f0, nf):
        """xyz[f, t, i] = (R_bexcl @ w_cum)[i] + t_bexcl[i]."""
        for i in range(3):
            V.tensor_mul(out=apx(tA, 0, (T, nf), (1, T)),
                         in0=apx(rx, f0 * 12 + 4 * i + 0, (12, nf), (0, T)),
                         in1=apx(w, f0 * 3 + 0, (3, nf), (F * 3, T)))
            V.tensor_mul(out=apx(tB, 0, (T, nf), (1, T)),
                         in0=apx(rx, f0 * 12 + 4 * i + 1, (12, nf), (0, T)),
                         in1=apx(w, f0 * 3 + 1, (3, nf), (F * 3, T)))
            V.tensor_add(out=apx(tA, 0, (1, nf * T)),
                         in0=apx(tA, 0, (1, nf * T)),
                         in1=apx(tB, 0, (1, nf * T)))
            V.tensor_mul(out=apx(tB, 0, (T, nf), (1, T)),
                         in0=apx(rx, f0 * 12 + 4 * i + 2, (12, nf), (0, T)),
                         in1=apx(w, f0 * 3 + 2, (3, nf), (F * 3, T)))
            V.tensor_add(out=apx(tB, 0, (T, nf), (1, T)),
                         in0=apx(tB, 0, (T, nf), (1, T)),
                         in1=apx(rx, f0 * 12 + 4 * i + 3, (12, nf), (0, T)))
            V.tensor_add(out=apx(xyz, f0 * T * 3 + i, (T * 3, nf), (3, T)),
                         in0=apx(tA, 0, (T, nf), (1, T)),
                         in1=apx(tB, 0, (T, nf), (1, T)))

    with tile.TileContext(nc) as tc:
      for _rep in range(repeat):
        with tc.tile_pool(name="main", bufs=1) as mp:
            X0 = mp.tile([P, T0 * F0 * 9], f32)
            dof0 = mp.tile([P, CHI * L0 * 4], f32)
            dof1 = mp.tile([P, CHI * L1 * 4], f32)
            w0 = mp.tile([P, T0 * F0 * 3], f32)
            tA0 = mp.tile([P, max(T0 * F0, F0 * 12)], f32)
            tB0 = mp.tile([P, max(T0 * F0, F0 * 12)], f32)
            tC0 = mp.tile([P, max(T0 * F0, F0 * 12)], f32)
            rx0 = mp.tile([P, F0 * 12], f32)
            rbr = mp.tile([P, CHI * 12], f32)
            a32 = mp.tile([P, CHI * 12], f32)
            jd = mp.tile([P, CHI * 9], f32)
            jang = mp.tile([P, CHI * 2 * 3], f32)
            jsin = mp.tile([P, CHI * 2 * 3], f32)
            jcos = mp.tile([P, CHI * 2 * 3], f32)
            re_ = mp.tile([P, CHI * 2 * 9], f32)
            rj = mp.tile([P, CHI * 9], f32)
            jtmp = mp.tile([P, CHI * 2 * 9], f32)
            halfpi = mp.tile([P, 1], f32)

            nc.sync.dma_start(out=jd[:], in_=jd_d[:])
            nc.vector.memset(halfpi[:], PI / 2)

            V = nc.vector
            SC = nc.scalar
            stt = V.scalar_tensor_tensor

            src = AP(b0_d, 0, [[L0 * 4, P], [P * L0 * 4, CHI], [1, L0 * 4]])
            dst = AP(dof0[:].tensor, dof0[:].offset,
                     [[dof0[:].ap[0][0], P], [L0 * 4, CHI], [1, L0 * 4]])
            nc.sync.dma_start(out=dst, in_=src)
            src = AP(b1_d, 0, [[L1 * 4, P], [P * L1 * 4, CHI], [1, L1 * 4]])
            dst = AP(dof1[:].tensor, dof1[:].offset,
                     [[dof1[:].ap[0][0], P], [L1 * 4, CHI], [1, L1 * 4]])
            nc.sync.dma_start(out=dst, in_=src)

            # ================= GEN 0: front + level-1 =================
            with tc.tile_pool(name="ptrig0", bufs=1) as pt:
                trig = {nm: pt.tile([P, CHI * L0], f32, name=f"t0_{nm}")
                        for nm in ("sa", "ca", "st", "ct", "apl", "aw")}

                def afix0(apl):
                    # chain position 1 has the jump as parent: alpha = phi_p
                    V.tensor_copy(out=apx(apl, 1, (L0, CHI)),
                                  in_=apx(dof0, 4, (L0 * 4, CHI)))

                emit_trig_fold(V, SC, dof0, trig, L0, halfpi, afix0)
                emit_init0(V, trig, X0, T0, F0, 0, F0)

                # ---- JUMP HT rotation for chain-start lanes (DVE) ----
                V.tensor_copy(out=jang[:], in_=apx(jd, 3, (9, CHI), (3, 2),
                                                   (1, 3)))
                V.add_range_wrap(out=jsin[:], in_=jang[:], shift=0.0,
                                 bound=PI, period=2 * PI)
                SC.activation(out=jsin[:], in_=jsin[:], func=SIN)
                V.add_range_wrap(out=jcos[:], in_=jang[:], shift=PI / 2,
                                 bound=PI, period=2 * PI)
                SC.activation(out=jcos[:], in_=jcos[:], func=SIN)

                CR = CHI * 2

                def sc_(tl, ang):
                    return apx(tl, ang, (3, CR))

                def re(e):
                    return apx(re_, e, (9, CR))

                def jt1(e):
                    return apx(jtmp, e, (9, CR))

                sa = lambda: sc_(jsin, 0)
                sb = lambda: sc_(jsin, 1)
                s_c = lambda: sc_(jsin, 2)
                ca = lambda: sc_(jcos, 0)
                cb = lambda: sc_(jcos, 1)
                c_c = lambda: sc_(jcos, 2)
                # R = Rz(c)Ry(b)Rx(a) per (chi, rot) lane
                V.tensor_mul(out=re(0), in0=c_c(), in1=cb())
                V.tensor_mul(out=jt1(0), in0=sb(), in1=sa())
                V.tensor_mul(out=jt1(1), in0=sb(), in1=ca())
                V.tensor_mul(out=jt1(2), in0=c_c(), in1=jt1(0))
                V.tensor_mul(out=jt1(3), in0=s_c(), in1=ca())
                V.tensor_sub(out=re(1), in0=jt1(2), in1=jt1(3))
                V.tensor_mul(out=jt1(2), in0=c_c(), in1=jt1(1))
                V.tensor_mul(out=jt1(3), in0=s_c(), in1=sa())
                V.tensor_add(out=re(2), in0=jt1(2), in1=jt1(3))
                V.tensor_mul(out=re(3), in0=s_c(), in1=cb())
                V.tensor_mul(out=jt1(2), in0=s_c(), in1=jt1(0))
                V.tensor_mul(out=jt1(3), in0=c_c(), in1=ca())
                V.tensor_add(out=re(4), in0=jt1(2), in1=jt1(3))
                V.tensor_mul(out=jt1(2), in0=s_c(), in1=jt1(1))
                V.tensor_mul(out=jt1(3), in0=c_c(), in1=sa())
                V.tensor_sub(out=re(5), in0=jt1(2), in1=jt1(3))
                V.tensor_scalar_mul(out=re(6), in0=sb(), scalar1=-1.0)
                V.tensor_mul(out=re(7), in0=cb(), in1=sa())
                V.tensor_mul(out=re(8), in0=cb(), in1=ca())
                # rj = R1 @ R2 (3x3), lanes = chi
                V.tensor_mul(
                    out=apx(rj, 0, (9, CHI), (3, 3), (1, 3)),
                    in0=apx(re_, 0, (18, CHI), (3, 3), (0, 3)),
                    in1=apx(re_, 9, (18, CHI), (0, 3), (1, 3)))
                V.tensor_mul(
                    out=apx(jtmp, 0, (9, CHI), (3, 3), (1, 3)),
                    in0=apx(re_, 1, (18, CHI), (3, 3), (0, 3)),
                    in1=apx(re_, 12, (18, CHI), (0, 3), (1, 3)))
                V.tensor_add(out=rj[:, : CHI * 9], in0=rj[:, : CHI * 9],
                             in1=jtmp[:, : CHI * 9])
                V.tensor_mul(
                    out=apx(jtmp, 0, (9, CHI), (3, 3), (1, 3)),
                    in0=apx(re_, 2, (18, CHI), (3, 3), (0, 3)),
                    in1=apx(re_, 15, (18, CHI), (0, 3), (1, 3)))
                V.tensor_add(out=rj[:, : CHI * 9], in0=rj[:, : CHI * 9],
                             in1=jtmp[:, : CHI * 9])
                # full jump 3x3 -> X0 slab 0, lane f=chi*J0 (j=0)
                V.tensor_copy(out=apx(X0, 0, (J0 * 9, CHI), (1, 9)),
                              in_=apx(rj, 0, (9, CHI), (1, 9)))

                emit_scan(V, stt, X0, trig, tA0, tB0, tC0, T0, F0, 0, F0)

            # ================= GEN 0: tail =================
            with tc.tile_pool(name="plev0", bufs=1) as pl0:
                bht0 = pl0.tile([P, F0 * 12], f32)
                lp2_0 = pl0.tile([P, CHI * S0 * (U0 + 1) * 12], f32)
                spx0 = pl0.tile([P, CHI * S0 * 12], f32)

                emit_w(V, X0, w0, dof0, tA0, tB0, T0, F0, 0, F0, F0)
                # jump translation overwrites w at (t=0, j=0) lanes
                V.tensor_copy(out=apx(w0, 0, (J0 * 3, CHI), (1, 3)),
                              in_=apx(jd, 0, (9, CHI), (1, 3)))
                emit_cumsum(V, w0, T0, F0, 0, F0)
                emit_bht(V, X0, w0, bht0, tA0, tB0, T0, F0, 0, F0, F0)
                emit_levels(V, SC, bht0, lp2_0, spx0, rx0, tA0, tB0, S0, U0)

                V.tensor_copy(out=apx(a32, 0, (12, CHI), (4, 2), (1, 3)),
                              in_=apx(X0, 32 * 9, (J0 * 9, CHI), (3, 2),
                                      (1, 3)))
                for dsti, (i1, i2), (i3, i4) in ((8, (1, 5), (2, 4)),
                                                 (9, (2, 3), (0, 5)),
                                                 (10, (0, 4), (1, 3))):
                    V.tensor_mul(out=apx(tA0, 0, (1, CHI)),
                                 in0=apx(X0, 32 * 9 + i1, (J0 * 9, CHI)),
                                 in1=apx(X0, 32 * 9 + i2, (J0 * 9, CHI)))
                    V.tensor_mul(out=apx(tB0, 0, (1, CHI)),
                                 in0=apx(X0, 32 * 9 + i3, (J0 * 9, CHI)),
                                 in1=apx(X0, 32 * 9 + i4, (J0 * 9, CHI)))
                    V.tensor_sub(out=apx(a32, dsti, (12, CHI)),
                                 in0=apx(tA0, 0, (1, CHI)),
                                 in1=apx(tB0, 0, (1, CHI)))
                V.tensor_copy(out=apx(a32, 3, (12, CHI), (4, 3)),
                              in_=apx(w0, 32 * 3, (J0 * 3, CHI), (1, 3)))
                compose_1d(V, CHI,
                           a_off=32 * 12, a_step=J0 * 12,
                           b_off=0, b_step=12,
                           o_off=0, o_step=12,
                           tA=tA0, tB=tB0,
                           a_tile=rx0, b_tile=a32, o_tile=rbr)

            # final transform; xyz aliases the dead X0 scan space
            emit_down(V, w0, rx0, X0, tA0, tB0, T0, F0, 0, F0)
            nc.sync.dma_start(
                out=AP(kin0_d, 0, [[F0 * T0 * 3, P], [1, F0 * T0 * 3]]),
                in_=apx(X0, 0, (1, F0 * T0 * 3)))

            # ================= GEN 1 =================
            with tc.tile_pool(name="pg1", bufs=1) as pg1:
                X1 = pg1.tile([P, T1 * F1 * 9], f32)
                w1 = pg1.tile([P, T1 * F1 * 3], f32)
                bht1 = pg1.tile([P, F1 * 12], f32)
                lp2_1 = pg1.tile([P, CHI * S1 * (U1 + 1) * 12], f32)
                spx1 = pg1.tile([P, CHI * S1 * 12], f32)
                rx1 = pg1.tile([P, F1 * 12], f32)
                trig1 = {nm: pg1.tile([P, CHI * L1], f32, name=f"t1_{nm}")
                         for nm in ("sa", "ca", "st", "ct", "apl", "aw")}

                def afix1(apl):
                    # branch position 0: alpha = phi_p + phi_c(gen0 atom 384)
                    V.tensor_add(out=apx(apl, 0, (L1, CHI)),
                                 in0=apx(dof1, 0, (L1 * 4, CHI)),
                                 in1=apx(dof0, 384 * 4 + 3, (L0 * 4, CHI)))

                emit_trig_fold(V, SC, dof1, trig1, L1, halfpi, afix1)
                emit_init0(V, trig1, X1, T1, F1, 0, F1)
                emit_scan(V, stt, X1, trig1, tA0, tB0, tC0, T1, F1, 0, F1)
                emit_w(V, X1, w1, dof1, tA0, tB0, T1, F1, 0, F1, F1)
                emit_cumsum(V, w1, T1, F1, 0, F1)
                emit_bht(V, X1, w1, bht1, tA0, tB0, T1, F1, 0, F1, F1)
                emit_levels(V, SC, bht1, lp2_1, spx1, rx1, tA0, tB0, S1, U1,
                            seed_rbr=rbr)
                emit_down(V, w1, rx1, X1, tA0, tB0, T1, F1, 0, F1)
                nc.sync.dma_start(
                    out=AP(kin1_d, 0, [[F1 * T1 * 3, P], [1, F1 * T1 * 3]]),
                    in_=apx(X1, 0, (1, F1 * T1 * 3)))

    nc.compile()
    return nc


def get_program(repeat=1):
    key = ("nc", repeat)
    if key not in _CACHE:
        _CACHE[key] = _build_program(repeat)
    return _CACHE[key]


# ------------------------------------------------------------------- host
def _shard_inputs(dofs, doftype):
    """Build the 8 per-core input maps (lane order (p, chi, j, t))."""
    in_maps = []
    chain_starts = 1 + np.arange(C0, dtype=np.int64) * L0
    jd_all = np.ascontiguousarray(dofs[chain_starts])       # [C0, 9]
    for core in range(NCORES):
        g0 = np.ascontiguousarray(
            dofs[1 + core * A0: 1 + (core + 1) * A0, :4])
        g1 = np.ascontiguousarray(
            dofs[BOFF + core * A1: BOFF + (core + 1) * A1, :4])
        jd = np.ascontiguousarray(
            jd_all[core * CH0:(core + 1) * CH0]
            .reshape(CHI, P, 9).transpose(1, 0, 2).reshape(P, CHI * 9))
        in_maps.append({"b0": g0, "b1": g1, "jd": jd})
    return in_maps


def _lane_ids(id_idx, core):
    """id_idx values of this core's atoms in device lane order (p, f, t)."""
    ids0 = (id_idx[core * A0:(core + 1) * A0]
            .reshape(CHI, P, L0).transpose(1, 0, 2).ravel())
    ids1 = (id_idx[BOFF - 1 + core * A1: BOFF - 1 + (core + 1) * A1]
            .reshape(CHI, P, L1).transpose(1, 0, 2).ravel())
    return ids0, ids1


def _structure_ok(doftype, gen0_paths, gen1_paths):
    chain_starts = 1 + np.arange(C0, dtype=np.int64) * L0
    g0 = np.concatenate(
        [np.zeros((C0, 1), np.int64), chain_starts[:, None] + np.arange(L0)],
        axis=1)
    if not np.array_equal(gen0_paths, g0.astype(gen0_paths.dtype)):
        return False
    branch_roots = chain_starts + L0 // 2
    g1 = np.concatenate(
        [branch_roots[:, None],
         BOFF + (np.arange(C1, dtype=np.int64) * L1)[:, None] + np.arange(L1)],
        axis=1)
    if not np.array_equal(gen1_paths, g1.astype(gen1_paths.dtype)):
        return False
    if doftype[0] != 0:
        return False
    if not np.all(doftype[chain_starts] == 1):
        return False
    dt = doftype.copy()
    dt[chain_starts] = 2
    if not np.all(dt[1:] == 2):
        return False
    return True


def _numpy_fallback(dofs, doftype, gen0_paths, gen1_paths, id_idx):
    """Exact numpy port of the reference (slow path, safety net)."""
    def rx(a):
        c, s = np.cos(a), np.sin(a)
        o, z = np.ones_like(a), np.zeros_like(a)
        return np.stack([np.stack([o, z, z, z], -1), np.stack([z, c, -s, z], -1),
                         np.stack([z, s, c, z], -1), np.stack([z, z, z, o], -1)], -2)

    def ry(a):
        c, s = np.cos(a), np.sin(a)
        o, z = np.ones_like(a), np.zeros_like(a)
        return np.stack([np.stack([c, z, s, z], -1), np.stack([z, o, z, z], -1),
                         np.stack([-s, z, c, z], -1), np.stack([z, z, z, o], -1)], -2)

    def rz(a):
        c, s = np.cos(a), np.sin(a)
        o, z = np.ones_like(a), np.zeros_like(a)
        return np.stack([np.stack([c, -s, z, z], -1), np.stack([s, c, z, z], -1),
                         np.stack([z, z, o, z], -1), np.stack([z, z, z, o], -1)], -2)

    def trans(x, y, z):
        o, zr = np.ones_like(x), np.zeros_like(x)
        return np.stack([np.stack([o, zr, zr, x], -1), np.stack([zr, o, zr, y], -1),
                         np.stack([zr, zr, o, z], -1), np.stack([zr, zr, zr, o], -1)], -2)

    dofs = dofs.astype(np.float32)
    phi_p, theta, d, phi_c = dofs[:, 0], dofs[:, 1], dofs[:, 2], dofs[:, 3]
    z = np.zeros_like(d)
    bond = rx(phi_p) @ rz(np.pi - theta) @ trans(d, z, z) @ rx(phi_c)
    rot = lambda a, b, c: rz(c) @ ry(b) @ rx(a)
    jump = (trans(dofs[:, 0], dofs[:, 1], dofs[:, 2])
            @ rot(dofs[:, 3], dofs[:, 4], dofs[:, 5])
            @ rot(dofs[:, 6], dofs[:, 7], dofs[:, 8]))
    eye = np.broadcast_to(np.eye(4, dtype=dofs.dtype), bond.shape)
    dt = doftype[:, None, None]
    hts = np.where(dt == 1, jump, np.where(dt == 2, bond, eye)).astype(np.float32)
    for paths in (gen0_paths, gen1_paths):
        seg = hts[paths]
        out = np.empty_like(seg)
        out[:, 0] = seg[:, 0]
        for i in range(1, seg.shape[1]):
            out[:, i] = out[:, i - 1] @ seg[:, i]
        hts[paths] = out
    kincoords = hts[:, :3, 3]
    coords = np.zeros((N - 1, 3), dtype=dofs.dtype)
    coords[np.asarray(id_idx)] = kincoords[1:]
    return coords


def kernel(dofs, doftype, gen0_paths, gen1_paths, id_idx):
    dofs = np.asarray(dofs, dtype=np.float32)
    doftype = np.asarray(doftype, dtype=np.int32)
    gen0_paths = np.asarray(gen0_paths)
    gen1_paths = np.asarray(gen1_paths)
    id_idx = np.asarray(id_idx, dtype=np.int32)

    if not _structure_ok(doftype, gen0_paths, gen1_paths):
        return _numpy_fallback(dofs, doftype, gen0_paths, gen1_paths, id_idx)

    from concourse.bass_utils import run_bass_kernel_spmd

    nc = get_program()
    in_maps = _shard_inputs(dofs, doftype)
    res = run_bass_kernel_spmd(nc, in_maps, core_ids=list(range(NCORES)))
    out = np.empty((N - 1, 3), dtype=np.float32)
    for core in range(NCORES):
        ids0, ids1 = _lane_ids(id_idx, core)
        out[ids0] = res.results[core]["kin0"].reshape(-1, 3)
        out[ids1] = res.results[core]["kin1"].reshape(-1, 3)
    return out


# revision 9
# speedup vs baseline: 1.5437x; 1.2281x over previous
"""Trainium2 Bass kernel for nn_KinematicOperation (kinematic tree forward).

Structure of the (deterministic) problem instance:
  - N = 1 + 2048*768 + 2048*256 atoms.
  - gen0: 2048 chains of 768 atoms rooted at the virtual root (identity HT);
    chain atoms are contiguous: chain c = atoms [1+c*768, 1+(c+1)*768).
  - gen1: 2048 branches of 256 atoms rooted mid-chain (gen0 chain c position
    384); branch atoms contiguous starting at boff = 1 + 2048*768.
  - Local HTs: BOND everywhere except a JUMP at each chain start; root = I.
  - Output: coords[id_idx[a-1]] = prefix_HT(a)[:3, 3] for atoms a = 1..N-1.

Sharding: core k owns gen0 chains [256k, 256(k+1)) and gen1 branches of the
same index range, so the branch-root HT handoff between generations stays
on-core and no collectives are needed.  Host pre-slices bond dof columns
(0..3 of 9) and gathers jump rows, shrinking input DMA.

Device algorithm per generation (fp32; rotations stored as 3x3 row-major,
translations separately):
  - ACT computes sin/cos (one DVE range-wrap per angle; cos as
    sin(pi/2 - |w|)); DVE assembles the local 3x3 rotations into SBUF.
  - 3-level blocked prefix scan along each chain:
      level1: rotation-only scan propagating ROWS 0,1 (6 elems) in place;
      translations via the NeRF identity local_t = d * col0(localR):
      t_glob(p) = sum_{q<=p} d_q * col0(R_glob_q), so in-block translations
      are prefix SUMS of w = d * col0(R_inblock) (col0 z-comp from a cross
      product), then level2/3 compose full 3x4 block HTs (tiny), and the
      final transform applies block-exclusive R,t to the in-block cumsums.
  - Output xyz written scatter-ready; host applies the id_idx permutation.
"""

import os
import sys

import numpy as np

for _p in ("/opt/trn_rl_repo", "/root/.axon_site/_ro/trn_rl_repo"):
    if os.path.isdir(_p) and _p not in sys.path:
        sys.path.insert(0, _p)

# ---------------------------------------------------------------- constants
C0, L0 = 2048, 768
C1, L1 = 2048, 256
N = 1 + C0 * L0 + C1 * L1
BOFF = 1 + C0 * L0
NCORES = 8
P = 128
CHI = 2                      # chains per partition (256 chains per core)
CH0 = C0 // NCORES
CH1 = C1 // NCORES
A0 = CH0 * L0                # 196608 gen0 atoms per core
A1 = CH1 * L1                # 65536 gen1 atoms per core

# block geometry: L = T*J,  J = S*U supers x blocks
T0, J0, S0, U0 = 12, 64, 8, 8
F0 = CHI * J0                # 128 block-lanes per partition
T1, J1, S1, U1 = 8, 32, 4, 8
F1 = CHI * J1                # 64

PI = float(np.pi)

_CACHE = {}


# ------------------------------------------------------------- device build
def _build_program(repeat=1):
    from concourse import bacc, mybir, tile
    from concourse.bass import AP

    f32 = mybir.dt.float32
    MUL = mybir.AluOpType.mult
    SUB = mybir.AluOpType.subtract
    SIN = mybir.ActivationFunctionType.Sin
    ABS = mybir.ActivationFunctionType.Abs

    nc = bacc.Bacc("TRN2", target_bir_lowering=False, debug=False)

    b0_d = nc.dram_tensor("b0", [A0, 4], f32, kind="ExternalInput")
    b1_d = nc.dram_tensor("b1", [A1, 4], f32, kind="ExternalInput")
    jd_d = nc.dram_tensor("jd", [P, CHI * 9], f32, kind="ExternalInput")
    kin0_d = nc.dram_tensor("kin0", [P, F0 * T0 * 3], f32, kind="ExternalOutput")
    kin1_d = nc.dram_tensor("kin1", [P, F1 * T1 * 3], f32, kind="ExternalOutput")

    def apx(tl, off, *dims):
        """AP over tile-AP `tl` at free-elem offset `off` with free dims
        [(step, count), ...] (full 128 partitions)."""
        t = tl[:] if not isinstance(tl, AP) else tl
        return AP(t.tensor, t.offset + off, [[t.ap[0][0], P]] + [list(d) for d in dims])

    def compose_1d(vec, lanes, a_off, a_step, b_off, b_step, o_off, o_step,
                   tA, tB, a_tile, b_tile, o_tile):
        """C = A @ B (3x4 HT compose, 12-elem row-major layout) over lanes."""
        for k, dst in ((0, tA), (1, tB)):
            vec.tensor_mul(
                out=apx(dst, 0, (12, lanes), (4, 3), (1, 4)),
                in0=apx(a_tile, a_off + k, (a_step, lanes), (4, 3), (0, 4)),
                in1=apx(b_tile, b_off + 4 * k, (b_step, lanes), (0, 3), (1, 4)),
            )
        vec.tensor_add(
            out=apx(tA, 0, (12, lanes), (1, 12)),
            in0=apx(tA, 0, (12, lanes), (1, 12)),
            in1=apx(tB, 0, (12, lanes), (1, 12)))
        vec.tensor_mul(
            out=apx(tB, 0, (12, lanes), (4, 3), (1, 4)),
            in0=apx(a_tile, a_off + 2, (a_step, lanes), (4, 3), (0, 4)),
            in1=apx(b_tile, b_off + 8, (b_step, lanes), (0, 3), (1, 4)),
        )
        vec.tensor_add(
            out=apx(o_tile, o_off, (o_step, lanes), (1, 12)),
            in0=apx(tA, 0, (12, lanes), (1, 12)),
            in1=apx(tB, 0, (12, lanes), (1, 12)),
        )
        vec.tensor_add(
            out=apx(o_tile, o_off + 3, (o_step, lanes), (4, 3)),
            in0=apx(o_tile, o_off + 3, (o_step, lanes), (4, 3)),
            in1=apx(a_tile, a_off + 3, (a_step, lanes), (4, 3)),
        )

    def excl_blocks(vec, CS, U, LPS, spx, lp2, rx, tA, tB):
        """rx[cs, u] = spx[cs] @ lp2[cs, u]  (exclusive block prefixes)."""
        for i in range(3):
            for k, dst in ((0, tA), (1, tB)):
                vec.tensor_mul(
                    out=apx(dst, 4 * i, (96, CS), (12, U), (1, 4)),
                    in0=apx(spx, 4 * i + k, (12, CS), (0, U), (0, 4)),
                    in1=apx(lp2, 4 * k, (LPS, CS), (12, U), (1, 4)))
            vec.tensor_add(
                out=apx(tA, 4 * i, (96, CS), (12, U), (1, 4)),
                in0=apx(tA, 4 * i, (96, CS), (12, U), (1, 4)),
                in1=apx(tB, 4 * i, (96, CS), (12, U), (1, 4)))
            vec.tensor_mul(
                out=apx(tB, 4 * i, (96, CS), (12, U), (1, 4)),
                in0=apx(spx, 4 * i + 2, (12, CS), (0, U), (0, 4)),
                in1=apx(lp2, 8, (LPS, CS), (12, U), (1, 4)))
            vec.tensor_add(
                out=apx(rx, 4 * i, (96, CS), (12, U), (1, 4)),
                in0=apx(tA, 4 * i, (96, CS), (12, U), (1, 4)),
                in1=apx(tB, 4 * i, (96, CS), (12, U), (1, 4)))
        vec.tensor_add(
            out=apx(rx, 3, (96, CS), (12, U), (4, 3)),
            in0=apx(rx, 3, (96, CS), (12, U), (4, 3)),
            in1=apx(spx, 3, (12, CS), (0, U), (4, 3)))

    # ---- generation emitters (engine-parameterized, f-lane-ranged) ----
    # Lane f = chi*J + j indexes a block; chain position = j*T + t, and the
    # per-atom source index chi*L + j*T + t == f*T + t, so every per-atom
    # phase uses 2-D (f, t) access patterns and splits at any f boundary.

    def emit_trig_fold(V, S, dof, trig, L, halfpi, alpha_fix):
        """Angle-folded trig: alpha_p = phi_c[p-1] + phi_p[p]; planes
        sa/ca = sin/cos(alpha), st/ct = sin/cos(theta).  One DVE wrap per
        angle (cos plane as scratch), cos = sin(pi/2 - |w|) on ACT.
        alpha_fix(apl) patches the chain-start alpha values."""
        apl, aw = trig["apl"], trig["aw"]
        V.tensor_add(out=apx(apl, 1, (L, CHI), (1, L - 1)),
                     in0=apx(dof, 4, (L * 4, CHI), (4, L - 1)),
                     in1=apx(dof, 3, (L * 4, CHI), (4, L - 1)))
        alpha_fix(apl)
        for src, cosn, sinn in ((apx(apl, 0, (L, CHI), (1, L)), "ca", "sa"),
                                (apx(dof, 1, (L * 4, CHI), (4, L)),
                                 "ct", "st")):
            V.add_range_wrap(out=trig[cosn][:], in_=src, shift=0.0,
                             bound=PI, period=2 * PI)
            S.activation(out=trig[sinn][:], in_=trig[cosn][:], func=SIN)
            S.activation(out=aw[:], in_=trig[cosn][:], func=ABS)
            S.activation(out=trig[cosn][:], in_=aw[:], func=SIN,
                         scale=-1.0, bias=halfpi[:])

    def emit_bond_fold(V, trig, X, T, F, f0, nf):
        """Folded local factor L' = Rx(alpha)Rz(pi-theta) for every atom:
        [[-ct, -st, 0], [ca*st, -ca*ct, -sa], [sa*st, -sa*ct, ca]]."""
        def tp(nm):
            return apx(trig[nm], f0 * T, (T, nf), (1, T))

        def xo(e):
            return apx(X, f0 * 9 + e, (9, nf), (F * 9, T))

        V.tensor_scalar_mul(out=xo(0), in0=tp("ct"), scalar1=-1.0)
        V.tensor_scalar_mul(out=xo(1), in0=tp("st"), scalar1=-1.0)
        V.memset(xo(2), 0.0)
        V.tensor_mul(out=xo(3), in0=tp("ca"), in1=tp("st"))
        V.tensor_mul(out=xo(4), in0=tp("ca"), in1=xo(0))
        V.tensor_scalar_mul(out=xo(5), in0=tp("sa"), scalar1=-1.0)
        V.tensor_mul(out=xo(6), in0=tp("sa"), in1=tp("st"))
        V.tensor_mul(out=xo(7), in0=tp("sa"), in1=xo(0))
        V.tensor_copy(out=xo(8), in_=tp("ca"))

    def emit_scan(V, X, tA, tB, T, F, f0, nf):
        """In-place in-block scan of rotation rows 0,1 for lanes [f0,f0+nf)
        (state in X slab t, elems 0..5; local row2 in elems 6..8 stays)."""
        for t in range(1, T):
            pb = (t - 1) * F * 9 + f0 * 9
            cb = t * F * 9 + f0 * 9
            V.tensor_mul(out=apx(tA, 0, (6, nf), (3, 2), (1, 3)),
                         in0=apx(X, pb + 0, (9, nf), (3, 2), (0, 3)),
                         in1=apx(X, cb + 0, (9, nf), (0, 2), (1, 3)))
            V.tensor_mul(out=apx(tB, 0, (6, nf), (3, 2), (1, 3)),
                         in0=apx(X, pb + 1, (9, nf), (3, 2), (0, 3)),
                         in1=apx(X, cb + 3, (9, nf), (0, 2), (1, 3)))
            V.tensor_add(out=apx(tA, 0, (1, 6 * nf)),
                         in0=apx(tA, 0, (1, 6 * nf)),
                         in1=apx(tB, 0, (1, 6 * nf)))
            V.tensor_mul(out=apx(tB, 0, (6, nf), (3, 2), (1, 3)),
                         in0=apx(X, pb + 2, (9, nf), (3, 2), (0, 3)),
                         in1=apx(X, cb + 6, (9, nf), (0, 2), (1, 3)))
            V.tensor_add(out=apx(X, cb, (9, nf), (3, 2), (1, 3)),
                         in0=apx(tA, 0, (6, nf), (3, 2), (1, 3)),
                         in1=apx(tB, 0, (6, nf), (3, 2), (1, 3)))

    def emit_w(V, X, w, dof, tA, tB, T, F, f0, nf, fw):
        """w[t, f, c] = d * col0(R_inblock) for lanes [f0, f0+nf); R20 via
        cross product kept in tA (local lane index f-f0, row width fw)."""
        d_ap = apx(dof, f0 * T * 4 + 2, (T * 4, nf), (4, T))
        V.tensor_mul(out=apx(tA, 0, (fw, T), (1, nf)),
                     in0=apx(X, f0 * 9 + 1, (F * 9, T), (9, nf)),
                     in1=apx(X, f0 * 9 + 5, (F * 9, T), (9, nf)))
        V.tensor_mul(out=apx(tB, 0, (fw, T), (1, nf)),
                     in0=apx(X, f0 * 9 + 2, (F * 9, T), (9, nf)),
                     in1=apx(X, f0 * 9 + 4, (F * 9, T), (9, nf)))
        V.tensor_sub(out=apx(tA, 0, (fw, T), (1, nf)),
                     in0=apx(tA, 0, (fw, T), (1, nf)),
                     in1=apx(tB, 0, (fw, T), (1, nf)))
        V.tensor_mul(out=apx(w, f0 * 3 + 2, (3, nf), (F * 3, T)),
                     in0=apx(tA, 0, (1, nf), (fw, T)),
                     in1=d_ap)
        V.tensor_mul(out=apx(w, f0 * 3 + 0, (3, nf), (F * 3, T)),
                     in0=apx(X, f0 * 9 + 0, (9, nf), (F * 9, T)),
                     in1=d_ap)
        V.tensor_mul(out=apx(w, f0 * 3 + 1, (3, nf), (F * 3, T)),
                     in0=apx(X, f0 * 9 + 3, (9, nf), (F * 9, T)),
                     in1=d_ap)

    def emit_cumsum(V, w, T, F, f0, nf):
        for t in range(1, T):
            V.tensor_add(out=apx(w, t * F * 3 + f0 * 3, (1, nf * 3)),
                         in0=apx(w, t * F * 3 + f0 * 3, (1, nf * 3)),
                         in1=apx(w, (t - 1) * F * 3 + f0 * 3, (1, nf * 3)))

    def emit_bht(V, X, w, bht, tA, tB, T, F, f0, nf, fw):
        """Assemble 12-elem (3x4 row-major) block-total HTs from the scan
        state at slab T-1 (+ row2 cross products; R20 reused from tA)."""
        base = (T - 1) * F * 9 + f0 * 9
        V.tensor_copy(out=apx(bht, f0 * 12, (12, nf), (4, 2), (1, 3)),
                      in_=apx(X, base, (9, nf), (3, 2), (1, 3)))
        V.tensor_copy(out=apx(bht, f0 * 12 + 8, (12, nf)),
                      in_=apx(tA, (T - 1) * fw, (1, nf)))
        # r21 = r02*r10 - r00*r12 ; r22 = r00*r11 - r01*r10
        for dst, (i1, i2), (i3, i4) in ((9, (2, 3), (0, 5)),
                                        (10, (0, 4), (1, 3))):
            V.tensor_mul(out=apx(tA, 0, (1, nf)),
                         in0=apx(X, base + i1, (9, nf)),
                         in1=apx(X, base + i2, (9, nf)))
            V.tensor_mul(out=apx(tB, 0, (1, nf)),
                         in0=apx(X, base + i3, (9, nf)),
                         in1=apx(X, base + i4, (9, nf)))
            V.tensor_sub(out=apx(bht, f0 * 12 + dst, (12, nf)),
                         in0=apx(tA, 0, (1, nf)),
                         in1=apx(tB, 0, (1, nf)))
        V.tensor_copy(out=apx(bht, f0 * 12 + 3, (12, nf), (4, 3)),
                      in_=apx(w, (T - 1) * F * 3 + f0 * 3, (3, nf), (1, 3)))

    def emit_levels(V, SC, bht, lp2, spx, rx, tA, tB, S, U, seed_rbr=None):
        """level2 (supers), level3 (exclusive over supers), excl_blocks."""
        CS = CHI * S
        LPS = (U + 1) * 12
        V.memset(lp2[:], 0.0)
        V.memset(apx(lp2, 0, (LPS, CS), (5, 3)), 1.0)
        SC.copy(out=apx(lp2, 12, (LPS, CS), (1, 12)),
                in_=apx(bht, 0, (U * 12, CS), (1, 12)))
        for u in range(1, U):
            compose_1d(V, CS,
                       a_off=u * 12, a_step=LPS,
                       b_off=u * 12, b_step=U * 12,
                       o_off=(u + 1) * 12, o_step=LPS,
                       tA=tA, tB=tB, a_tile=lp2, b_tile=bht, o_tile=lp2)
        if seed_rbr is None:
            V.memset(spx[:], 0.0)
            V.memset(apx(spx, 0, (S * 12, CHI), (5, 3)), 1.0)
        else:
            V.tensor_copy(out=apx(spx, 0, (S * 12, CHI), (1, 12)),
                          in_=apx(seed_rbr, 0, (12, CHI), (1, 12)))
        for s in range(1, S):
            compose_1d(V, CHI,
                       a_off=(s - 1) * 12, a_step=S * 12,
                       b_off=(s - 1) * LPS + U * 12, b_step=S * LPS,
                       o_off=s * 12, o_step=S * 12,
                       tA=tA, tB=tB, a_tile=spx, b_tile=lp2, o_tile=spx)
        excl_blocks(V, CS, U, LPS, spx, lp2, rx, tA, tB)

    def emit_down(V, w, rx, xyz, tA, tB, T, F, f0, nf):
        """xyz[f, t, i] = (R_bexcl @ w_cum)[i] + t_bexcl[i]."""
        for i in range(3):
            V.tensor_mul(out=apx(tA, 0, (T, nf), (1, T)),
                         in0=apx(rx, f0 * 12 + 4 * i + 0, (12, nf), (0, T)),
                         in1=apx(w, f0 * 3 + 0, (3, nf), (F * 3, T)))
            V.tensor_mul(out=apx(tB, 0, (T, nf), (1, T)),
                         in0=apx(rx, f0 * 12 + 4 * i + 1, (12, nf), (0, T)),
                         in1=apx(w, f0 * 3 + 1, (3, nf), (F * 3, T)))
            V.tensor_add(out=apx(tA, 0, (1, nf * T)),
                         in0=apx(tA, 0, (1, nf * T)),
                         in1=apx(tB, 0, (1, nf * T)))
            V.tensor_mul(out=apx(tB, 0, (T, nf), (1, T)),
                         in0=apx(rx, f0 * 12 + 4 * i + 2, (12, nf), (0, T)),
                         in1=apx(w, f0 * 3 + 2, (3, nf), (F * 3, T)))
            V.tensor_add(out=apx(tB, 0, (T, nf), (1, T)),
                         in0=apx(tB, 0, (T, nf), (1, T)),
                         in1=apx(rx, f0 * 12 + 4 * i + 3, (12, nf), (0, T)))
            V.tensor_add(out=apx(xyz, f0 * T * 3 + i, (T * 3, nf), (3, T)),
                         in0=apx(tA, 0, (T, nf), (1, T)),
                         in1=apx(tB, 0, (T, nf), (1, T)))

    with tile.TileContext(nc) as tc:
      for _rep in range(repeat):
        with tc.tile_pool(name="main", bufs=1) as mp:
            X0 = mp.tile([P, T0 * F0 * 9], f32)
            dof0 = mp.tile([P, CHI * L0 * 4], f32)
            dof1 = mp.tile([P, CHI * L1 * 4], f32)
            w0 = mp.tile([P, T0 * F0 * 3], f32)
            tA0 = mp.tile([P, max(T0 * F0, F0 * 12)], f32)
            tB0 = mp.tile([P, max(T0 * F0, F0 * 12)], f32)
            tC0 = mp.tile([P, max(T0 * F0, F0 * 12)], f32)
            rx0 = mp.tile([P, F0 * 12], f32)
            rbr = mp.tile([P, CHI * 12], f32)
            a32 = mp.tile([P, CHI * 12], f32)
            jd = mp.tile([P, CHI * 9], f32)
            jang = mp.tile([P, CHI * 2 * 3], f32)
            jsin = mp.tile([P, CHI * 2 * 3], f32)
            jcos = mp.tile([P, CHI * 2 * 3], f32)
            re_ = mp.tile([P, CHI * 2 * 9], f32)
            rj = mp.tile([P, CHI * 9], f32)
            jtmp = mp.tile([P, CHI * 2 * 9], f32)
            halfpi = mp.tile([P, 1], f32)

            nc.sync.dma_start(out=jd[:], in_=jd_d[:])
            nc.vector.memset(halfpi[:], PI / 2)

            V = nc.vector
            SC = nc.scalar
            stt = V.scalar_tensor_tensor

            src = AP(b0_d, 0, [[L0 * 4, P], [P * L0 * 4, CHI], [1, L0 * 4]])
            dst = AP(dof0[:].tensor, dof0[:].offset,
                     [[dof0[:].ap[0][0], P], [L0 * 4, CHI], [1, L0 * 4]])
            nc.sync.dma_start(out=dst, in_=src)
            src = AP(b1_d, 0, [[L1 * 4, P], [P * L1 * 4, CHI], [1, L1 * 4]])
            dst = AP(dof1[:].tensor, dof1[:].offset,
                     [[dof1[:].ap[0][0], P], [L1 * 4, CHI], [1, L1 * 4]])
            nc.sync.dma_start(out=dst, in_=src)

            # ================= GEN 0: front + level-1 =================
            with tc.tile_pool(name="ptrig0", bufs=1) as pt:
                trig = {nm: pt.tile([P, CHI * L0], f32, name=f"t0_{nm}")
                        for nm in ("sa", "ca", "st", "ct", "apl", "aw")}

                def afix0(apl):
                    # chain position 1 has the jump as parent: alpha = phi_p
                    V.tensor_copy(out=apx(apl, 1, (L0, CHI)),
                                  in_=apx(dof0, 4, (L0 * 4, CHI)))

                emit_trig_fold(V, SC, dof0, trig, L0, halfpi, afix0)
                emit_bond_fold(V, trig, X0, T0, F0, 0, F0)

                # ---- JUMP HT rotation for chain-start lanes (DVE) ----
                V.tensor_copy(out=jang[:], in_=apx(jd, 3, (9, CHI), (3, 2),
                                                   (1, 3)))
                V.add_range_wrap(out=jsin[:], in_=jang[:], shift=0.0,
                                 bound=PI, period=2 * PI)
                SC.activation(out=jsin[:], in_=jsin[:], func=SIN)
                V.add_range_wrap(out=jcos[:], in_=jang[:], shift=PI / 2,
                                 bound=PI, period=2 * PI)
                SC.activation(out=jcos[:], in_=jcos[:], func=SIN)

                CR = CHI * 2

                def sc_(tl, ang):
                    return apx(tl, ang, (3, CR))

                def re(e):
                    return apx(re_, e, (9, CR))

                def jt1(e):
                    return apx(jtmp, e, (9, CR))

                sa = lambda: sc_(jsin, 0)
                sb = lambda: sc_(jsin, 1)
                s_c = lambda: sc_(jsin, 2)
                ca = lambda: sc_(jcos, 0)
                cb = lambda: sc_(jcos, 1)
                c_c = lambda: sc_(jcos, 2)
                # R = Rz(c)Ry(b)Rx(a) per (chi, rot) lane
                V.tensor_mul(out=re(0), in0=c_c(), in1=cb())
                V.tensor_mul(out=jt1(0), in0=sb(), in1=sa())
                V.tensor_mul(out=jt1(1), in0=sb(), in1=ca())
                V.tensor_mul(out=jt1(2), in0=c_c(), in1=jt1(0))
                V.tensor_mul(out=jt1(3), in0=s_c(), in1=ca())
                V.tensor_sub(out=re(1), in0=jt1(2), in1=jt1(3))
                V.tensor_mul(out=jt1(2), in0=c_c(), in1=jt1(1))
                V.tensor_mul(out=jt1(3), in0=s_c(), in1=sa())
                V.tensor_add(out=re(2), in0=jt1(2), in1=jt1(3))
                V.tensor_mul(out=re(3), in0=s_c(), in1=cb())
                V.tensor_mul(out=jt1(2), in0=s_c(), in1=jt1(0))
                V.tensor_mul(out=jt1(3), in0=c_c(), in1=ca())
                V.tensor_add(out=re(4), in0=jt1(2), in1=jt1(3))
                V.tensor_mul(out=jt1(2), in0=s_c(), in1=jt1(1))
                V.tensor_mul(out=jt1(3), in0=c_c(), in1=sa())
                V.tensor_sub(out=re(5), in0=jt1(2), in1=jt1(3))
                V.tensor_scalar_mul(out=re(6), in0=sb(), scalar1=-1.0)
                V.tensor_mul(out=re(7), in0=cb(), in1=sa())
                V.tensor_mul(out=re(8), in0=cb(), in1=ca())
                # rj = R1 @ R2 (3x3), lanes = chi
                V.tensor_mul(
                    out=apx(rj, 0, (9, CHI), (3, 3), (1, 3)),
                    in0=apx(re_, 0, (18, CHI), (3, 3), (0, 3)),
                    in1=apx(re_, 9, (18, CHI), (0, 3), (1, 3)))
                V.tensor_mul(
                    out=apx(jtmp, 0, (9, CHI), (3, 3), (1, 3)),
                    in0=apx(re_, 1, (18, CHI), (3, 3), (0, 3)),
                    in1=apx(re_, 12, (18, CHI), (0, 3), (1, 3)))
                V.tensor_add(out=rj[:, : CHI * 9], in0=rj[:, : CHI * 9],
                             in1=jtmp[:, : CHI * 9])
                V.tensor_mul(
                    out=apx(jtmp, 0, (9, CHI), (3, 3), (1, 3)),
                    in0=apx(re_, 2, (18, CHI), (3, 3), (0, 3)),
                    in1=apx(re_, 15, (18, CHI), (0, 3), (1, 3)))
                V.tensor_add(out=rj[:, : CHI * 9], in0=rj[:, : CHI * 9],
                             in1=jtmp[:, : CHI * 9])
                # full jump 3x3 -> X0 slab 0, lane f=chi*J0 (j=0)
                V.tensor_copy(out=apx(X0, 0, (J0 * 9, CHI), (1, 9)),
                              in_=apx(rj, 0, (9, CHI), (1, 9)))

                emit_scan(V, X0, tA0, tB0, T0, F0, 0, F0)

            # ================= GEN 0: tail =================
            with tc.tile_pool(name="plev0", bufs=1) as pl0:
                bht0 = pl0.tile([P, F0 * 12], f32)
                lp2_0 = pl0.tile([P, CHI * S0 * (U0 + 1) * 12], f32)
                spx0 = pl0.tile([P, CHI * S0 * 12], f32)

                emit_w(V, X0, w0, dof0, tA0, tB0, T0, F0, 0, F0, F0)
                # jump translation overwrites w at (t=0, j=0) lanes
                V.tensor_copy(out=apx(w0, 0, (J0 * 3, CHI), (1, 3)),
                              in_=apx(jd, 0, (9, CHI), (1, 3)))
                emit_cumsum(V, w0, T0, F0, 0, F0)
                emit_bht(V, X0, w0, bht0, tA0, tB0, T0, F0, 0, F0, F0)
                emit_levels(V, SC, bht0, lp2_0, spx0, rx0, tA0, tB0, S0, U0)

                V.tensor_copy(out=apx(a32, 0, (12, CHI), (4, 2), (1, 3)),
                              in_=apx(X0, 32 * 9, (J0 * 9, CHI), (3, 2),
                                      (1, 3)))
                for dsti, (i1, i2), (i3, i4) in ((8, (1, 5), (2, 4)),
                                                 (9, (2, 3), (0, 5)),
                                                 (10, (0, 4), (1, 3))):
                    V.tensor_mul(out=apx(tA0, 0, (1, CHI)),
                                 in0=apx(X0, 32 * 9 + i1, (J0 * 9, CHI)),
                                 in1=apx(X0, 32 * 9 + i2, (J0 * 9, CHI)))
                    V.tensor_mul(out=apx(tB0, 0, (1, CHI)),
                                 in0=apx(X0, 32 * 9 + i3, (J0 * 9, CHI)),
                                 in1=apx(X0, 32 * 9 + i4, (J0 * 9, CHI)))
                    V.tensor_sub(out=apx(a32, dsti, (12, CHI)),
                                 in0=apx(tA0, 0, (1, CHI)),
                                 in1=apx(tB0, 0, (1, CHI)))
                V.tensor_copy(out=apx(a32, 3, (12, CHI), (4, 3)),
                              in_=apx(w0, 32 * 3, (J0 * 3, CHI), (1, 3)))
                compose_1d(V, CHI,
                           a_off=32 * 12, a_step=J0 * 12,
                           b_off=0, b_step=12,
                           o_off=0, o_step=12,
                           tA=tA0, tB=tB0,
                           a_tile=rx0, b_tile=a32, o_tile=rbr)

            # final transform; xyz aliases the dead X0 scan space
            emit_down(V, w0, rx0, X0, tA0, tB0, T0, F0, 0, F0)
            nc.sync.dma_start(
                out=AP(kin0_d, 0, [[F0 * T0 * 3, P], [1, F0 * T0 * 3]]),
                in_=apx(X0, 0, (1, F0 * T0 * 3)))

            # ================= GEN 1 =================
            with tc.tile_pool(name="pg1", bufs=1) as pg1:
                X1 = pg1.tile([P, T1 * F1 * 9], f32)
                w1 = pg1.tile([P, T1 * F1 * 3], f32)
                bht1 = pg1.tile([P, F1 * 12], f32)
                lp2_1 = pg1.tile([P, CHI * S1 * (U1 + 1) * 12], f32)
                spx1 = pg1.tile([P, CHI * S1 * 12], f32)
                rx1 = pg1.tile([P, F1 * 12], f32)
                trig1 = {nm: pg1.tile([P, CHI * L1], f32, name=f"t1_{nm}")
                         for nm in ("sa", "ca", "st", "ct", "apl", "aw")}

                def afix1(apl):
                    # branch position 0: alpha = phi_p + phi_c(gen0 atom 384)
                    V.tensor_add(out=apx(apl, 0, (L1, CHI)),
                                 in0=apx(dof1, 0, (L1 * 4, CHI)),
                                 in1=apx(dof0, 384 * 4 + 3, (L0 * 4, CHI)))

                emit_trig_fold(V, SC, dof1, trig1, L1, halfpi, afix1)
                emit_bond_fold(V, trig1, X1, T1, F1, 0, F1)
                emit_scan(V, X1, tA0, tB0, T1, F1, 0, F1)
                emit_w(V, X1, w1, dof1, tA0, tB0, T1, F1, 0, F1, F1)
                emit_cumsum(V, w1, T1, F1, 0, F1)
                emit_bht(V, X1, w1, bht1, tA0, tB0, T1, F1, 0, F1, F1)
                emit_levels(V, SC, bht1, lp2_1, spx1, rx1, tA0, tB0, S1, U1,
                            seed_rbr=rbr)
                emit_down(V, w1, rx1, X1, tA0, tB0, T1, F1, 0, F1)
                nc.sync.dma_start(
                    out=AP(kin1_d, 0, [[F1 * T1 * 3, P], [1, F1 * T1 * 3]]),
                    in_=apx(X1, 0, (1, F1 * T1 * 3)))

    nc.compile()
    return nc


def get_program(repeat=1):
    key = ("nc", repeat)
    if key not in _CACHE:
        _CACHE[key] = _build_program(repeat)
    return _CACHE[key]


# ------------------------------------------------------------------- host
def _shard_inputs(dofs, doftype):
    """Build the 8 per-core input maps (lane order (p, chi, j, t))."""
    in_maps = []
    chain_starts = 1 + np.arange(C0, dtype=np.int64) * L0
    jd_all = np.ascontiguousarray(dofs[chain_starts])       # [C0, 9]
    for core in range(NCORES):
        g0 = np.ascontiguousarray(
            dofs[1 + core * A0: 1 + (core + 1) * A0, :4])
        g1 = np.ascontiguousarray(
            dofs[BOFF + core * A1: BOFF + (core + 1) * A1, :4])
        jd = np.ascontiguousarray(
            jd_all[core * CH0:(core + 1) * CH0]
            .reshape(CHI, P, 9).transpose(1, 0, 2).reshape(P, CHI * 9))
        in_maps.append({"b0": g0, "b1": g1, "jd": jd})
    return in_maps


def _lane_ids(id_idx, core):
    """id_idx values of this core's atoms in device lane order (p, f, t)."""
    ids0 = (id_idx[core * A0:(core + 1) * A0]
            .reshape(CHI, P, L0).transpose(1, 0, 2).ravel())
    ids1 = (id_idx[BOFF - 1 + core * A1: BOFF - 1 + (core + 1) * A1]
            .reshape(CHI, P, L1).transpose(1, 0, 2).ravel())
    return ids0, ids1


def _structure_ok(doftype, gen0_paths, gen1_paths):
    chain_starts = 1 + np.arange(C0, dtype=np.int64) * L0
    g0 = np.concatenate(
        [np.zeros((C0, 1), np.int64), chain_starts[:, None] + np.arange(L0)],
        axis=1)
    if not np.array_equal(gen0_paths, g0.astype(gen0_paths.dtype)):
        return False
    branch_roots = chain_starts + L0 // 2
    g1 = np.concatenate(
        [branch_roots[:, None],
         BOFF + (np.arange(C1, dtype=np.int64) * L1)[:, None] + np.arange(L1)],
        axis=1)
    if not np.array_equal(gen1_paths, g1.astype(gen1_paths.dtype)):
        return False
    if doftype[0] != 0:
        return False
    if not np.all(doftype[chain_starts] == 1):
        return False
    dt = doftype.copy()
    dt[chain_starts] = 2
    if not np.all(dt[1:] == 2):
        return False
    return True


def _numpy_fallback(dofs, doftype, gen0_paths, gen1_paths, id_idx):
    """Exact numpy port of the reference (slow path, safety net)."""
    def rx(a):
        c, s = np.cos(a), np.sin(a)
        o, z = np.ones_like(a), np.zeros_like(a)
        return np.stack([np.stack([o, z, z, z], -1), np.stack([z, c, -s, z], -1),
                         np.stack([z, s, c, z], -1), np.stack([z, z, z, o], -1)], -2)

    def ry(a):
        c, s = np.cos(a), np.sin(a)
        o, z = np.ones_like(a), np.zeros_like(a)
        return np.stack([np.stack([c, z, s, z], -1), np.stack([z, o, z, z], -1),
                         np.stack([-s, z, c, z], -1), np.stack([z, z, z, o], -1)], -2)

    def rz(a):
        c, s = np.cos(a), np.sin(a)
        o, z = np.ones_like(a), np.zeros_like(a)
        return np.stack([np.stack([c, -s, z, z], -1), np.stack([s, c, z, z], -1),
                         np.stack([z, z, o, z], -1), np.stack([z, z, z, o], -1)], -2)

    def trans(x, y, z):
        o, zr = np.ones_like(x), np.zeros_like(x)
        return np.stack([np.stack([o, zr, zr, x], -1), np.stack([zr, o, zr, y], -1),
                         np.stack([zr, zr, o, z], -1), np.stack([zr, zr, zr, o], -1)], -2)

    dofs = dofs.astype(np.float32)
    phi_p, theta, d, phi_c = dofs[:, 0], dofs[:, 1], dofs[:, 2], dofs[:, 3]
    z = np.zeros_like(d)
    bond = rx(phi_p) @ rz(np.pi - theta) @ trans(d, z, z) @ rx(phi_c)
    rot = lambda a, b, c: rz(c) @ ry(b) @ rx(a)
    jump = (trans(dofs[:, 0], dofs[:, 1], dofs[:, 2])
            @ rot(dofs[:, 3], dofs[:, 4], dofs[:, 5])
            @ rot(dofs[:, 6], dofs[:, 7], dofs[:, 8]))
    eye = np.broadcast_to(np.eye(4, dtype=dofs.dtype), bond.shape)
    dt = doftype[:, None, None]
    hts = np.where(dt == 1, jump, np.where(dt == 2, bond, eye)).astype(np.float32)
    for paths in (gen0_paths, gen1_paths):
        seg = hts[paths]
        out = np.empty_like(seg)
        out[:, 0] = seg[:, 0]
        for i in range(1, seg.shape[1]):
            out[:, i] = out[:, i - 1] @ seg[:, i]
        hts[paths] = out
    kincoords = hts[:, :3, 3]
    coords = np.zeros((N - 1, 3), dtype=dofs.dtype)
    coords[np.asarray(id_idx)] = kincoords[1:]
    return coords


def kernel(dofs, doftype, gen0_paths, gen1_paths, id_idx):
    dofs = np.asarray(dofs, dtype=np.float32)
    doftype = np.asarray(doftype, dtype=np.int32)
    gen0_paths = np.asarray(gen0_paths)
    gen1_paths = np.asarray(gen1_paths)
    id_idx = np.asarray(id_idx, dtype=np.int32)

    if not _structure_ok(doftype, gen0_paths, gen1_paths):
        return _numpy_fallback(dofs, doftype, gen0_paths, gen1_paths, id_idx)

    from concourse.bass_utils import run_bass_kernel_spmd

    nc = get_program()
    in_maps = _shard_inputs(dofs, doftype)
    res = run_bass_kernel_spmd(nc, in_maps, core_ids=list(range(NCORES)))
    out = np.empty((N - 1, 3), dtype=np.float32)
    for core in range(NCORES):
        ids0, ids1 = _lane_ids(id_idx, core)
        out[ids0] = res.results[core]["kin0"].reshape(-1, 3)
        out[ids1] = res.results[core]["kin1"].reshape(-1, 3)
    return out


# revision 10
# speedup vs baseline: 1.5678x; 1.0156x over previous
"""Trainium2 Bass kernel for nn_KinematicOperation (kinematic tree forward).

Structure of the (deterministic) problem instance:
  - N = 1 + 2048*768 + 2048*256 atoms.
  - gen0: 2048 chains of 768 atoms rooted at the virtual root (identity HT);
    chain atoms are contiguous: chain c = atoms [1+c*768, 1+(c+1)*768).
  - gen1: 2048 branches of 256 atoms rooted mid-chain (gen0 chain c position
    384); branch atoms contiguous starting at boff = 1 + 2048*768.
  - Local HTs: BOND everywhere except a JUMP at each chain start; root = I.
  - Output: coords[id_idx[a-1]] = prefix_HT(a)[:3, 3] for atoms a = 1..N-1.

Sharding: core k owns gen0 chains [256k, 256(k+1)) and gen1 branches of the
same index range, so the branch-root HT handoff between generations stays
on-core and no collectives are needed.  Host pre-slices bond dof columns
(0..3 of 9) and gathers jump rows, shrinking input DMA.

Device algorithm per generation (fp32; rotations stored as 3x3 row-major,
translations separately):
  - ACT computes sin/cos (one DVE range-wrap per angle; cos as
    sin(pi/2 - |w|)); DVE assembles the local 3x3 rotations into SBUF.
  - 3-level blocked prefix scan along each chain:
      level1: rotation-only scan propagating ROWS 0,1 (6 elems) in place;
      translations via the NeRF identity local_t = d * col0(localR):
      t_glob(p) = sum_{q<=p} d_q * col0(R_glob_q), so in-block translations
      are prefix SUMS of w = d * col0(R_inblock) (col0 z-comp from a cross
      product), then level2/3 compose full 3x4 block HTs (tiny), and the
      final transform applies block-exclusive R,t to the in-block cumsums.
  - Output xyz written scatter-ready; host applies the id_idx permutation.
"""

import os
import sys

import numpy as np

for _p in ("/opt/trn_rl_repo", "/root/.axon_site/_ro/trn_rl_repo"):
    if os.path.isdir(_p) and _p not in sys.path:
        sys.path.insert(0, _p)

# ---------------------------------------------------------------- constants
C0, L0 = 2048, 768
C1, L1 = 2048, 256
N = 1 + C0 * L0 + C1 * L1
BOFF = 1 + C0 * L0
NCORES = 8
P = 128
CHI = 2                      # chains per partition (256 chains per core)
CH0 = C0 // NCORES
CH1 = C1 // NCORES
A0 = CH0 * L0                # 196608 gen0 atoms per core
A1 = CH1 * L1                # 65536 gen1 atoms per core

# block geometry: L = T*J,  J = S*U supers x blocks
T0, J0, S0, U0 = 12, 64, 8, 8
F0 = CHI * J0                # 128 block-lanes per partition
T1, J1, S1, U1 = 8, 32, 4, 8
F1 = CHI * J1                # 64

PI = float(np.pi)

_CACHE = {}


# ------------------------------------------------------------- device build
def _build_program(repeat=1):
    from concourse import bacc, mybir, tile
    from concourse.bass import AP

    f32 = mybir.dt.float32
    MUL = mybir.AluOpType.mult
    SUB = mybir.AluOpType.subtract
    SIN = mybir.ActivationFunctionType.Sin
    ABS = mybir.ActivationFunctionType.Abs

    nc = bacc.Bacc("TRN2", target_bir_lowering=False, debug=False)

    b0_d = nc.dram_tensor("b0", [A0, 4], f32, kind="ExternalInput")
    b1_d = nc.dram_tensor("b1", [A1, 4], f32, kind="ExternalInput")
    jd_d = nc.dram_tensor("jd", [P, CHI * 9], f32, kind="ExternalInput")
    kin0_d = nc.dram_tensor("kin0", [P, F0 * T0 * 3], f32, kind="ExternalOutput")
    kin1_d = nc.dram_tensor("kin1", [P, F1 * T1 * 3], f32, kind="ExternalOutput")

    def apx(tl, off, *dims):
        """AP over tile-AP `tl` at free-elem offset `off` with free dims
        [(step, count), ...] (full 128 partitions)."""
        t = tl[:] if not isinstance(tl, AP) else tl
        return AP(t.tensor, t.offset + off, [[t.ap[0][0], P]] + [list(d) for d in dims])

    def compose_1d(vec, lanes, a_off, a_step, b_off, b_step, o_off, o_step,
                   tA, tB, a_tile, b_tile, o_tile):
        """C = A @ B (3x4 HT compose, 12-elem row-major layout) over lanes."""
        for k, dst in ((0, tA), (1, tB)):
            vec.tensor_mul(
                out=apx(dst, 0, (12, lanes), (4, 3), (1, 4)),
                in0=apx(a_tile, a_off + k, (a_step, lanes), (4, 3), (0, 4)),
                in1=apx(b_tile, b_off + 4 * k, (b_step, lanes), (0, 3), (1, 4)),
            )
        vec.tensor_add(
            out=apx(tA, 0, (12, lanes), (1, 12)),
            in0=apx(tA, 0, (12, lanes), (1, 12)),
            in1=apx(tB, 0, (12, lanes), (1, 12)))
        vec.tensor_mul(
            out=apx(tB, 0, (12, lanes), (4, 3), (1, 4)),
            in0=apx(a_tile, a_off + 2, (a_step, lanes), (4, 3), (0, 4)),
            in1=apx(b_tile, b_off + 8, (b_step, lanes), (0, 3), (1, 4)),
        )
        vec.tensor_add(
            out=apx(o_tile, o_off, (o_step, lanes), (1, 12)),
            in0=apx(tA, 0, (12, lanes), (1, 12)),
            in1=apx(tB, 0, (12, lanes), (1, 12)),
        )
        vec.tensor_add(
            out=apx(o_tile, o_off + 3, (o_step, lanes), (4, 3)),
            in0=apx(o_tile, o_off + 3, (o_step, lanes), (4, 3)),
            in1=apx(a_tile, a_off + 3, (a_step, lanes), (4, 3)),
        )

    def excl_blocks(vec, CS, U, LPS, spx, lp2, rx, tA, tB):
        """rx[cs, u] = spx[cs] @ lp2[cs, u]  (exclusive block prefixes)."""
        for i in range(3):
            for k, dst in ((0, tA), (1, tB)):
                vec.tensor_mul(
                    out=apx(dst, 4 * i, (96, CS), (12, U), (1, 4)),
                    in0=apx(spx, 4 * i + k, (12, CS), (0, U), (0, 4)),
                    in1=apx(lp2, 4 * k, (LPS, CS), (12, U), (1, 4)))
            vec.tensor_add(
                out=apx(tA, 4 * i, (96, CS), (12, U), (1, 4)),
                in0=apx(tA, 4 * i, (96, CS), (12, U), (1, 4)),
                in1=apx(tB, 4 * i, (96, CS), (12, U), (1, 4)))
            vec.tensor_mul(
                out=apx(tB, 4 * i, (96, CS), (12, U), (1, 4)),
                in0=apx(spx, 4 * i + 2, (12, CS), (0, U), (0, 4)),
                in1=apx(lp2, 8, (LPS, CS), (12, U), (1, 4)))
            vec.tensor_add(
                out=apx(rx, 4 * i, (96, CS), (12, U), (1, 4)),
                in0=apx(tA, 4 * i, (96, CS), (12, U), (1, 4)),
                in1=apx(tB, 4 * i, (96, CS), (12, U), (1, 4)))
        vec.tensor_add(
            out=apx(rx, 3, (96, CS), (12, U), (4, 3)),
            in0=apx(rx, 3, (96, CS), (12, U), (4, 3)),
            in1=apx(spx, 3, (12, CS), (0, U), (4, 3)))

    # ---- generation emitters (engine-parameterized, f-lane-ranged) ----
    # Lane f = chi*J + j indexes a block; chain position = j*T + t, and the
    # per-atom source index chi*L + j*T + t == f*T + t, so every per-atom
    # phase uses 2-D (f, t) access patterns and splits at any f boundary.

    def emit_trig_fold(V, S, dof, trig, L, halfpi, alpha_fix):
        """Angle-folded trig: alpha_p = phi_c[p-1] + phi_p[p]; planes
        sa/ca = sin/cos(alpha), st/ct = sin/cos(theta).  One DVE wrap per
        angle (cos plane as scratch), cos = sin(pi/2 - |w|) on ACT.
        alpha_fix(apl) patches the chain-start alpha values."""
        apl, aw = trig["apl"], trig["aw"]
        V.tensor_add(out=apx(apl, 1, (L, CHI), (1, L - 1)),
                     in0=apx(dof, 4, (L * 4, CHI), (4, L - 1)),
                     in1=apx(dof, 3, (L * 4, CHI), (4, L - 1)))
        alpha_fix(apl)
        for src, cosn, sinn in ((apx(apl, 0, (L, CHI), (1, L)), "ca", "sa"),
                                (apx(dof, 1, (L * 4, CHI), (4, L)),
                                 "ct", "st")):
            V.add_range_wrap(out=trig[cosn][:], in_=src, shift=0.0,
                             bound=PI, period=2 * PI)
            S.activation(out=trig[sinn][:], in_=trig[cosn][:], func=SIN)
            S.activation(out=aw[:], in_=trig[cosn][:], func=ABS)
            S.activation(out=trig[cosn][:], in_=aw[:], func=SIN,
                         scale=-1.0, bias=halfpi[:])

    def emit_bond_fold(V, trig, X, T, F, f0, nf):
        """Folded local factor L' = Rx(alpha)Rz(pi-theta) for every atom:
        [[-ct, -st, 0], [ca*st, -ca*ct, -sa], [sa*st, -sa*ct, ca]]."""
        def tp(nm):
            return apx(trig[nm], f0 * T, (T, nf), (1, T))

        def xo(e):
            return apx(X, f0 * 9 + e, (9, nf), (F * 9, T))

        V.tensor_scalar_mul(out=xo(0), in0=tp("ct"), scalar1=-1.0)
        V.tensor_scalar_mul(out=xo(1), in0=tp("st"), scalar1=-1.0)
        V.memset(xo(2), 0.0)
        V.tensor_mul(out=xo(3), in0=tp("ca"), in1=tp("st"))
        V.tensor_mul(out=xo(4), in0=tp("ca"), in1=xo(0))
        V.tensor_scalar_mul(out=xo(5), in0=tp("sa"), scalar1=-1.0)
        V.tensor_mul(out=xo(6), in0=tp("sa"), in1=tp("st"))
        V.tensor_mul(out=xo(7), in0=tp("sa"), in1=xo(0))
        V.tensor_copy(out=xo(8), in_=tp("ca"))

    def emit_scan(V, X, tA, tB, tC, T, F, f0, nf):
        """In-place in-block scan of rotation rows 0,1 for lanes [f0,f0+nf)
        (state in X slab t, elems 0..5; local row2 in elems 6..8 stays)."""
        for t in range(1, T):
            pb = (t - 1) * F * 9 + f0 * 9
            cb = t * F * 9 + f0 * 9
            V.tensor_mul(out=apx(tA, 0, (6, nf), (3, 2), (1, 3)),
                         in0=apx(X, pb + 0, (9, nf), (3, 2), (0, 3)),
                         in1=apx(X, cb + 0, (9, nf), (0, 2), (1, 3)))
            V.tensor_mul(out=apx(tB, 0, (6, nf), (3, 2), (1, 3)),
                         in0=apx(X, pb + 1, (9, nf), (3, 2), (0, 3)),
                         in1=apx(X, cb + 3, (9, nf), (0, 2), (1, 3)))
            V.tensor_mul(out=apx(tC, 0, (6, nf), (3, 2), (1, 3)),
                         in0=apx(X, pb + 2, (9, nf), (3, 2), (0, 3)),
                         in1=apx(X, cb + 6, (9, nf), (0, 2), (1, 3)))
            V.tensor_add(out=apx(tA, 0, (1, 6 * nf)),
                         in0=apx(tA, 0, (1, 6 * nf)),
                         in1=apx(tB, 0, (1, 6 * nf)))
            V.tensor_add(out=apx(X, cb, (9, nf), (3, 2), (1, 3)),
                         in0=apx(tA, 0, (6, nf), (3, 2), (1, 3)),
                         in1=apx(tC, 0, (6, nf), (3, 2), (1, 3)))

    def emit_w(V, X, w, dof, tA, tB, T, F, f0, nf, fw):
        """w[t, f, c] = d * col0(R_inblock) for lanes [f0, f0+nf); R20 via
        cross product kept in tA (local lane index f-f0, row width fw)."""
        d_ap = apx(dof, f0 * T * 4 + 2, (T * 4, nf), (4, T))
        V.tensor_mul(out=apx(tA, 0, (fw, T), (1, nf)),
                     in0=apx(X, f0 * 9 + 1, (F * 9, T), (9, nf)),
                     in1=apx(X, f0 * 9 + 5, (F * 9, T), (9, nf)))
        V.tensor_mul(out=apx(tB, 0, (fw, T), (1, nf)),
                     in0=apx(X, f0 * 9 + 2, (F * 9, T), (9, nf)),
                     in1=apx(X, f0 * 9 + 4, (F * 9, T), (9, nf)))
        V.tensor_sub(out=apx(tA, 0, (fw, T), (1, nf)),
                     in0=apx(tA, 0, (fw, T), (1, nf)),
                     in1=apx(tB, 0, (fw, T), (1, nf)))
        V.tensor_mul(out=apx(w, f0 * 3 + 2, (3, nf), (F * 3, T)),
                     in0=apx(tA, 0, (1, nf), (fw, T)),
                     in1=d_ap)
        V.tensor_mul(out=apx(w, f0 * 3 + 0, (3, nf), (F * 3, T)),
                     in0=apx(X, f0 * 9 + 0, (9, nf), (F * 9, T)),
                     in1=d_ap)
        V.tensor_mul(out=apx(w, f0 * 3 + 1, (3, nf), (F * 3, T)),
                     in0=apx(X, f0 * 9 + 3, (9, nf), (F * 9, T)),
                     in1=d_ap)

    def emit_cumsum(V, w, T, F, f0, nf):
        for t in range(1, T):
            V.tensor_add(out=apx(w, t * F * 3 + f0 * 3, (1, nf * 3)),
                         in0=apx(w, t * F * 3 + f0 * 3, (1, nf * 3)),
                         in1=apx(w, (t - 1) * F * 3 + f0 * 3, (1, nf * 3)))

    def emit_bht(V, SC, X, w, bht, tA, tB, T, F, f0, nf, fw):
        """Assemble 12-elem (3x4 row-major) block-total HTs from the scan
        state at slab T-1 (+ row2 cross products; R20 reused from tA)."""
        base = (T - 1) * F * 9 + f0 * 9
        SC.copy(out=apx(bht, f0 * 12, (12, nf), (4, 2), (1, 3)),
                in_=apx(X, base, (9, nf), (3, 2), (1, 3)))
        SC.copy(out=apx(bht, f0 * 12 + 8, (12, nf)),
                in_=apx(tA, (T - 1) * fw, (1, nf)))
        # r21 = r02*r10 - r00*r12 ; r22 = r00*r11 - r01*r10
        for dst, (i1, i2), (i3, i4) in ((9, (2, 3), (0, 5)),
                                        (10, (0, 4), (1, 3))):
            V.tensor_mul(out=apx(tA, 0, (1, nf)),
                         in0=apx(X, base + i1, (9, nf)),
                         in1=apx(X, base + i2, (9, nf)))
            V.tensor_mul(out=apx(tB, 0, (1, nf)),
                         in0=apx(X, base + i3, (9, nf)),
                         in1=apx(X, base + i4, (9, nf)))
            V.tensor_sub(out=apx(bht, f0 * 12 + dst, (12, nf)),
                         in0=apx(tA, 0, (1, nf)),
                         in1=apx(tB, 0, (1, nf)))
        SC.copy(out=apx(bht, f0 * 12 + 3, (12, nf), (4, 3)),
                in_=apx(w, (T - 1) * F * 3 + f0 * 3, (3, nf), (1, 3)))

    def emit_levels(V, SC, bht, lp2, spx, rx, tA, tB, S, U, seed_rbr=None):
        """level2 (supers), level3 (exclusive over supers), excl_blocks."""
        CS = CHI * S
        LPS = (U + 1) * 12
        V.memset(lp2[:], 0.0)
        V.memset(apx(lp2, 0, (LPS, CS), (5, 3)), 1.0)
        SC.copy(out=apx(lp2, 12, (LPS, CS), (1, 12)),
                in_=apx(bht, 0, (U * 12, CS), (1, 12)))
        for u in range(1, U):
            compose_1d(V, CS,
                       a_off=u * 12, a_step=LPS,
                       b_off=u * 12, b_step=U * 12,
                       o_off=(u + 1) * 12, o_step=LPS,
                       tA=tA, tB=tB, a_tile=lp2, b_tile=bht, o_tile=lp2)
        if seed_rbr is None:
            V.memset(spx[:], 0.0)
            V.memset(apx(spx, 0, (S * 12, CHI), (5, 3)), 1.0)
        else:
            SC.copy(out=apx(spx, 0, (S * 12, CHI), (1, 12)),
                    in_=apx(seed_rbr, 0, (12, CHI), (1, 12)))
        for s in range(1, S):
            compose_1d(V, CHI,
                       a_off=(s - 1) * 12, a_step=S * 12,
                       b_off=(s - 1) * LPS + U * 12, b_step=S * LPS,
                       o_off=s * 12, o_step=S * 12,
                       tA=tA, tB=tB, a_tile=spx, b_tile=lp2, o_tile=spx)
        excl_blocks(V, CS, U, LPS, spx, lp2, rx, tA, tB)

    def emit_down(V, w, rx, xyz, tA, tB, T, F, f0, nf):
        """xyz[f, t, i] = (R_bexcl @ w_cum)[i] + t_bexcl[i]."""
        for i in range(3):
            V.tensor_mul(out=apx(tA, 0, (T, nf), (1, T)),
                         in0=apx(rx, f0 * 12 + 4 * i + 0, (12, nf), (0, T)),
                         in1=apx(w, f0 * 3 + 0, (3, nf), (F * 3, T)))
            V.tensor_mul(out=apx(tB, 0, (T, nf), (1, T)),
                         in0=apx(rx, f0 * 12 + 4 * i + 1, (12, nf), (0, T)),
                         in1=apx(w, f0 * 3 + 1, (3, nf), (F * 3, T)))
            V.tensor_add(out=apx(tA, 0, (1, nf * T)),
                         in0=apx(tA, 0, (1, nf * T)),
                         in1=apx(tB, 0, (1, nf * T)))
            V.tensor_mul(out=apx(tB, 0, (T, nf), (1, T)),
                         in0=apx(rx, f0 * 12 + 4 * i + 2, (12, nf), (0, T)),
                         in1=apx(w, f0 * 3 + 2, (3, nf), (F * 3, T)))
            V.tensor_add(out=apx(tB, 0, (T, nf), (1, T)),
                         in0=apx(tB, 0, (T, nf), (1, T)),
                         in1=apx(rx, f0 * 12 + 4 * i + 3, (12, nf), (0, T)))
            V.tensor_add(out=apx(xyz, f0 * T * 3 + i, (T * 3, nf), (3, T)),
                         in0=apx(tA, 0, (T, nf), (1, T)),
                         in1=apx(tB, 0, (T, nf), (1, T)))

    with tile.TileContext(nc) as tc:
      for _rep in range(repeat):
        with tc.tile_pool(name="main", bufs=1) as mp:
            X0 = mp.tile([P, T0 * F0 * 9], f32)
            dof0 = mp.tile([P, CHI * L0 * 4], f32)
            dof1 = mp.tile([P, CHI * L1 * 4], f32)
            w0 = mp.tile([P, T0 * F0 * 3], f32)
            tA0 = mp.tile([P, max(T0 * F0, F0 * 12)], f32)
            tB0 = mp.tile([P, max(T0 * F0, F0 * 12)], f32)
            tC0 = mp.tile([P, max(T0 * F0, F0 * 12)], f32)
            rx0 = mp.tile([P, F0 * 12], f32)
            rbr = mp.tile([P, CHI * 12], f32)
            a32 = mp.tile([P, CHI * 12], f32)
            jd = mp.tile([P, CHI * 9], f32)
            jang = mp.tile([P, CHI * 2 * 3], f32)
            jsin = mp.tile([P, CHI * 2 * 3], f32)
            jcos = mp.tile([P, CHI * 2 * 3], f32)
            re_ = mp.tile([P, CHI * 2 * 9], f32)
            rj = mp.tile([P, CHI * 9], f32)
            jtmp = mp.tile([P, CHI * 2 * 9], f32)
            halfpi = mp.tile([P, 1], f32)
            trig1 = {nm: mp.tile([P, CHI * L1], f32, name=f"t1_{nm}")
                     for nm in ("sa", "ca", "st", "ct", "apl", "aw")}

            nc.sync.dma_start(out=jd[:], in_=jd_d[:])
            nc.vector.memset(halfpi[:], PI / 2)

            V = nc.vector
            SC = nc.scalar
            stt = V.scalar_tensor_tensor

            src = AP(b0_d, 0, [[L0 * 4, P], [P * L0 * 4, CHI], [1, L0 * 4]])
            dst = AP(dof0[:].tensor, dof0[:].offset,
                     [[dof0[:].ap[0][0], P], [L0 * 4, CHI], [1, L0 * 4]])
            nc.sync.dma_start(out=dst, in_=src)
            src = AP(b1_d, 0, [[L1 * 4, P], [P * L1 * 4, CHI], [1, L1 * 4]])
            dst = AP(dof1[:].tensor, dof1[:].offset,
                     [[dof1[:].ap[0][0], P], [L1 * 4, CHI], [1, L1 * 4]])
            nc.sync.dma_start(out=dst, in_=src)

            # ================= GEN 0: front + level-1 =================
            with tc.tile_pool(name="ptrig0", bufs=1) as pt:
                trig = {nm: pt.tile([P, CHI * L0], f32, name=f"t0_{nm}")
                        for nm in ("sa", "ca", "st", "ct", "apl", "aw")}

                def afix0(apl):
                    # chain position 1 has the jump as parent: alpha = phi_p
                    V.tensor_copy(out=apx(apl, 1, (L0, CHI)),
                                  in_=apx(dof0, 4, (L0 * 4, CHI)))

                emit_trig_fold(V, SC, dof0, trig, L0, halfpi, afix0)
                emit_bond_fold(V, trig, X0, T0, F0, 0, F0)

                # ---- JUMP HT rotation for chain-start lanes (DVE) ----
                V.tensor_copy(out=jang[:], in_=apx(jd, 3, (9, CHI), (3, 2),
                                                   (1, 3)))
                V.add_range_wrap(out=jsin[:], in_=jang[:], shift=0.0,
                                 bound=PI, period=2 * PI)
                SC.activation(out=jsin[:], in_=jsin[:], func=SIN)
                V.add_range_wrap(out=jcos[:], in_=jang[:], shift=PI / 2,
                                 bound=PI, period=2 * PI)
                SC.activation(out=jcos[:], in_=jcos[:], func=SIN)

                CR = CHI * 2

                def sc_(tl, ang):
                    return apx(tl, ang, (3, CR))

                def re(e):
                    return apx(re_, e, (9, CR))

                def jt1(e):
                    return apx(jtmp, e, (9, CR))

                sa = lambda: sc_(jsin, 0)
                sb = lambda: sc_(jsin, 1)
                s_c = lambda: sc_(jsin, 2)
                ca = lambda: sc_(jcos, 0)
                cb = lambda: sc_(jcos, 1)
                c_c = lambda: sc_(jcos, 2)
                # R = Rz(c)Ry(b)Rx(a) per (chi, rot) lane
                V.tensor_mul(out=re(0), in0=c_c(), in1=cb())
                V.tensor_mul(out=jt1(0), in0=sb(), in1=sa())
                V.tensor_mul(out=jt1(1), in0=sb(), in1=ca())
                V.tensor_mul(out=jt1(2), in0=c_c(), in1=jt1(0))
                V.tensor_mul(out=jt1(3), in0=s_c(), in1=ca())
                V.tensor_sub(out=re(1), in0=jt1(2), in1=jt1(3))
                V.tensor_mul(out=jt1(2), in0=c_c(), in1=jt1(1))
                V.tensor_mul(out=jt1(3), in0=s_c(), in1=sa())
                V.tensor_add(out=re(2), in0=jt1(2), in1=jt1(3))
                V.tensor_mul(out=re(3), in0=s_c(), in1=cb())
                V.tensor_mul(out=jt1(2), in0=s_c(), in1=jt1(0))
                V.tensor_mul(out=jt1(3), in0=c_c(), in1=ca())
                V.tensor_add(out=re(4), in0=jt1(2), in1=jt1(3))
                V.tensor_mul(out=jt1(2), in0=s_c(), in1=jt1(1))
                V.tensor_mul(out=jt1(3), in0=c_c(), in1=sa())
                V.tensor_sub(out=re(5), in0=jt1(2), in1=jt1(3))
                V.tensor_scalar_mul(out=re(6), in0=sb(), scalar1=-1.0)
                V.tensor_mul(out=re(7), in0=cb(), in1=sa())
                V.tensor_mul(out=re(8), in0=cb(), in1=ca())
                # rj = R1 @ R2 (3x3), lanes = chi
                V.tensor_mul(
                    out=apx(rj, 0, (9, CHI), (3, 3), (1, 3)),
                    in0=apx(re_, 0, (18, CHI), (3, 3), (0, 3)),
                    in1=apx(re_, 9, (18, CHI), (0, 3), (1, 3)))
                V.tensor_mul(
                    out=apx(jtmp, 0, (9, CHI), (3, 3), (1, 3)),
                    in0=apx(re_, 1, (18, CHI), (3, 3), (0, 3)),
                    in1=apx(re_, 12, (18, CHI), (0, 3), (1, 3)))
                V.tensor_add(out=rj[:, : CHI * 9], in0=rj[:, : CHI * 9],
                             in1=jtmp[:, : CHI * 9])
                V.tensor_mul(
                    out=apx(jtmp, 0, (9, CHI), (3, 3), (1, 3)),
                    in0=apx(re_, 2, (18, CHI), (3, 3), (0, 3)),
                    in1=apx(re_, 15, (18, CHI), (0, 3), (1, 3)))
                V.tensor_add(out=rj[:, : CHI * 9], in0=rj[:, : CHI * 9],
                             in1=jtmp[:, : CHI * 9])
                # full jump 3x3 -> X0 slab 0, lane f=chi*J0 (j=0)
                V.tensor_copy(out=apx(X0, 0, (J0 * 9, CHI), (1, 9)),
                              in_=apx(rj, 0, (9, CHI), (1, 9)))

                emit_scan(V, X0, tA0, tB0, tC0, T0, F0, 0, F0)

                # gen1 trig early: ACT computes it under the gen0 scan
                def afix1(apl):
                    # branch position 0: alpha = phi_p + phi_c(gen0 atom 384)
                    V.tensor_add(out=apx(apl, 0, (L1, CHI)),
                                 in0=apx(dof1, 0, (L1 * 4, CHI)),
                                 in1=apx(dof0, 384 * 4 + 3, (L0 * 4, CHI)))

                emit_trig_fold(V, SC, dof1, trig1, L1, halfpi, afix1)

            # ================= GEN 0: tail =================
            with tc.tile_pool(name="plev0", bufs=1) as pl0:
                bht0 = pl0.tile([P, F0 * 12], f32)
                lp2_0 = pl0.tile([P, CHI * S0 * (U0 + 1) * 12], f32)
                spx0 = pl0.tile([P, CHI * S0 * 12], f32)

                emit_w(V, X0, w0, dof0, tA0, tB0, T0, F0, 0, F0, F0)
                # jump translation overwrites w at (t=0, j=0) lanes
                V.tensor_copy(out=apx(w0, 0, (J0 * 3, CHI), (1, 3)),
                              in_=apx(jd, 0, (9, CHI), (1, 3)))
                emit_cumsum(V, w0, T0, F0, 0, F0)
                emit_bht(V, SC, X0, w0, bht0, tA0, tB0, T0, F0, 0, F0, F0)
                emit_levels(V, SC, bht0, lp2_0, spx0, rx0, tA0, tB0, S0, U0)

                V.tensor_copy(out=apx(a32, 0, (12, CHI), (4, 2), (1, 3)),
                              in_=apx(X0, 32 * 9, (J0 * 9, CHI), (3, 2),
                                      (1, 3)))
                for dsti, (i1, i2), (i3, i4) in ((8, (1, 5), (2, 4)),
                                                 (9, (2, 3), (0, 5)),
                                                 (10, (0, 4), (1, 3))):
                    V.tensor_mul(out=apx(tA0, 0, (1, CHI)),
                                 in0=apx(X0, 32 * 9 + i1, (J0 * 9, CHI)),
                                 in1=apx(X0, 32 * 9 + i2, (J0 * 9, CHI)))
                    V.tensor_mul(out=apx(tB0, 0, (1, CHI)),
                                 in0=apx(X0, 32 * 9 + i3, (J0 * 9, CHI)),
                                 in1=apx(X0, 32 * 9 + i4, (J0 * 9, CHI)))
                    V.tensor_sub(out=apx(a32, dsti, (12, CHI)),
                                 in0=apx(tA0, 0, (1, CHI)),
                                 in1=apx(tB0, 0, (1, CHI)))
                V.tensor_copy(out=apx(a32, 3, (12, CHI), (4, 3)),
                              in_=apx(w0, 32 * 3, (J0 * 3, CHI), (1, 3)))
                compose_1d(V, CHI,
                           a_off=32 * 12, a_step=J0 * 12,
                           b_off=0, b_step=12,
                           o_off=0, o_step=12,
                           tA=tA0, tB=tB0,
                           a_tile=rx0, b_tile=a32, o_tile=rbr)

            # final transform; xyz aliases the dead X0 scan space
            emit_down(V, w0, rx0, X0, tA0, tB0, T0, F0, 0, F0)
            nc.sync.dma_start(
                out=AP(kin0_d, 0, [[F0 * T0 * 3, P], [1, F0 * T0 * 3]]),
                in_=apx(X0, 0, (1, F0 * T0 * 3)))

            # ================= GEN 1 =================
            with tc.tile_pool(name="pg1", bufs=1) as pg1:
                X1 = pg1.tile([P, T1 * F1 * 9], f32)
                w1 = pg1.tile([P, T1 * F1 * 3], f32)
                bht1 = pg1.tile([P, F1 * 12], f32)
                lp2_1 = pg1.tile([P, CHI * S1 * (U1 + 1) * 12], f32)
                spx1 = pg1.tile([P, CHI * S1 * 12], f32)
                rx1 = pg1.tile([P, F1 * 12], f32)
                emit_bond_fold(V, trig1, X1, T1, F1, 0, F1)
                emit_scan(V, X1, tA0, tB0, tC0, T1, F1, 0, F1)
                emit_w(V, X1, w1, dof1, tA0, tB0, T1, F1, 0, F1, F1)
                emit_cumsum(V, w1, T1, F1, 0, F1)
                emit_bht(V, SC, X1, w1, bht1, tA0, tB0, T1, F1, 0, F1, F1)
                emit_levels(V, SC, bht1, lp2_1, spx1, rx1, tA0, tB0, S1, U1,
                            seed_rbr=rbr)
                emit_down(V, w1, rx1, X1, tA0, tB0, T1, F1, 0, F1)
                nc.sync.dma_start(
                    out=AP(kin1_d, 0, [[F1 * T1 * 3, P], [1, F1 * T1 * 3]]),
                    in_=apx(X1, 0, (1, F1 * T1 * 3)))

    nc.compile()
    return nc


def get_program(repeat=1):
    key = ("nc", repeat)
    if key not in _CACHE:
        _CACHE[key] = _build_program(repeat)
    return _CACHE[key]


# ------------------------------------------------------------------- host
def _shard_inputs(dofs, doftype):
    """Build the 8 per-core input maps (lane order (p, chi, j, t))."""
    in_maps = []
    chain_starts = 1 + np.arange(C0, dtype=np.int64) * L0
    jd_all = np.ascontiguousarray(dofs[chain_starts])       # [C0, 9]
    for core in range(NCORES):
        g0 = np.ascontiguousarray(
            dofs[1 + core * A0: 1 + (core + 1) * A0, :4])
        g1 = np.ascontiguousarray(
            dofs[BOFF + core * A1: BOFF + (core + 1) * A1, :4])
        jd = np.ascontiguousarray(
            jd_all[core * CH0:(core + 1) * CH0]
            .reshape(CHI, P, 9).transpose(1, 0, 2).reshape(P, CHI * 9))
        in_maps.append({"b0": g0, "b1": g1, "jd": jd})
    return in_maps


def _lane_ids(id_idx, core):
    """id_idx values of this core's atoms in device lane order (p, f, t)."""
    ids0 = (id_idx[core * A0:(core + 1) * A0]
            .reshape(CHI, P, L0).transpose(1, 0, 2).ravel())
    ids1 = (id_idx[BOFF - 1 + core * A1: BOFF - 1 + (core + 1) * A1]
            .reshape(CHI, P, L1).transpose(1, 0, 2).ravel())
    return ids0, ids1


def _structure_ok(doftype, gen0_paths, gen1_paths):
    chain_starts = 1 + np.arange(C0, dtype=np.int64) * L0
    g0 = np.concatenate(
        [np.zeros((C0, 1), np.int64), chain_starts[:, None] + np.arange(L0)],
        axis=1)
    if not np.array_equal(gen0_paths, g0.astype(gen0_paths.dtype)):
        return False
    branch_roots = chain_starts + L0 // 2
    g1 = np.concatenate(
        [branch_roots[:, None],
         BOFF + (np.arange(C1, dtype=np.int64) * L1)[:, None] + np.arange(L1)],
        axis=1)
    if not np.array_equal(gen1_paths, g1.astype(gen1_paths.dtype)):
        return False
    if doftype[0] != 0:
        return False
    if not np.all(doftype[chain_starts] == 1):
        return False
    dt = doftype.copy()
    dt[chain_starts] = 2
    if not np.all(dt[1:] == 2):
        return False
    return True


def _numpy_fallback(dofs, doftype, gen0_paths, gen1_paths, id_idx):
    """Exact numpy port of the reference (slow path, safety net)."""
    def rx(a):
        c, s = np.cos(a), np.sin(a)
        o, z = np.ones_like(a), np.zeros_like(a)
        return np.stack([np.stack([o, z, z, z], -1), np.stack([z, c, -s, z], -1),
                         np.stack([z, s, c, z], -1), np.stack([z, z, z, o], -1)], -2)

    def ry(a):
        c, s = np.cos(a), np.sin(a)
        o, z = np.ones_like(a), np.zeros_like(a)
        return np.stack([np.stack([c, z, s, z], -1), np.stack([z, o, z, z], -1),
                         np.stack([-s, z, c, z], -1), np.stack([z, z, z, o], -1)], -2)

    def rz(a):
        c, s = np.cos(a), np.sin(a)
        o, z = np.ones_like(a), np.zeros_like(a)
        return np.stack([np.stack([c, -s, z, z], -1), np.stack([s, c, z, z], -1),
                         np.stack([z, z, o, z], -1), np.stack([z, z, z, o], -1)], -2)

    def trans(x, y, z):
        o, zr = np.ones_like(x), np.zeros_like(x)
        return np.stack([np.stack([o, zr, zr, x], -1), np.stack([zr, o, zr, y], -1),
                         np.stack([zr, zr, o, z], -1), np.stack([zr, zr, zr, o], -1)], -2)

    dofs = dofs.astype(np.float32)
    phi_p, theta, d, phi_c = dofs[:, 0], dofs[:, 1], dofs[:, 2], dofs[:, 3]
    z = np.zeros_like(d)
    bond = rx(phi_p) @ rz(np.pi - theta) @ trans(d, z, z) @ rx(phi_c)
    rot = lambda a, b, c: rz(c) @ ry(b) @ rx(a)
    jump = (trans(dofs[:, 0], dofs[:, 1], dofs[:, 2])
            @ rot(dofs[:, 3], dofs[:, 4], dofs[:, 5])
            @ rot(dofs[:, 6], dofs[:, 7], dofs[:, 8]))
    eye = np.broadcast_to(np.eye(4, dtype=dofs.dtype), bond.shape)
    dt = doftype[:, None, None]
    hts = np.where(dt == 1, jump, np.where(dt == 2, bond, eye)).astype(np.float32)
    for paths in (gen0_paths, gen1_paths):
        seg = hts[paths]
        out = np.empty_like(seg)
        out[:, 0] = seg[:, 0]
        for i in range(1, seg.shape[1]):
            out[:, i] = out[:, i - 1] @ seg[:, i]
        hts[paths] = out
    kincoords = hts[:, :3, 3]
    coords = np.zeros((N - 1, 3), dtype=dofs.dtype)
    coords[np.asarray(id_idx)] = kincoords[1:]
    return coords


def kernel(dofs, doftype, gen0_paths, gen1_paths, id_idx):
    dofs = np.asarray(dofs, dtype=np.float32)
    doftype = np.asarray(doftype, dtype=np.int32)
    gen0_paths = np.asarray(gen0_paths)
    gen1_paths = np.asarray(gen1_paths)
    id_idx = np.asarray(id_idx, dtype=np.int32)

    if not _structure_ok(doftype, gen0_paths, gen1_paths):
        return _numpy_fallback(dofs, doftype, gen0_paths, gen1_paths, id_idx)

    from concourse.bass_utils import run_bass_kernel_spmd

    nc = get_program()
    in_maps = _shard_inputs(dofs, doftype)
    res = run_bass_kernel_spmd(nc, in_maps, core_ids=list(range(NCORES)))
    out = np.empty((N - 1, 3), dtype=np.float32)
    for core in range(NCORES):
        ids0, ids1 = _lane_ids(id_idx, core)
        out[ids0] = res.results[core]["kin0"].reshape(-1, 3)
        out[ids1] = res.results[core]["kin1"].reshape(-1, 3)
    return out
